# revision 12
# baseline (speedup 1.0000x reference)
"""Trainium2 Bass kernel for a 2-layer GCN (gnn_message_passing) — v3.

Strategy (8 NeuronCores, SPMD). Profile-driven rebuild of v2:
  - The v2 bottleneck was GPSIMD: dma_gather costs ~5-8ns/index + ~3us
    fixed of Pool-engine time (Q7 descriptor-gen), so v3 uses far fewer,
    far larger gather calls: K_CALLMAX=4096 (one call per run instead of
    ~8) -> 70 calls/layer instead of 266. Calls >1024 indices require
    single_packet=False: with single_packet=True the whole per-engine
    descriptor stream coalesces into ONE DMA packet, and 1024 idx x 256B
    / 16 engines = 16KB is the packet-size ceiling (bigger crashes).
  - PSUM: one bank per open dest block (HW allows only ONE live matmul
    accumulation group per bank — interleaving two groups in one bank
    drops accumulations; verified by micro-test), so SEPT=7 + head.
  - Tables stored bf16 padded to 256B rows ([N,128] bf16, features in
    cols 0..63). dma_gather descriptors then move 128B/row instead of
    256B (elem_size=64, elem_step=128 -> stride 256B satisfies the HW
    stride%256 constraint; the bass-level elem_size_bytes%256 assert is a
    transpose-mode restriction, bypassed by emitting InstDMAGatherAnt
    directly). Halves gather HBM traffic and kills the fp32->bf16 cast.
  - Scatter masks (weighted one-hot [slot x dest] bf16) are built on DVE
    (tensor_scalar is_eq*mult) and optionally a fraction on the idle
    Activation engine via two activations: y=Abs(iota-c) then
    mask=Relu(w - w*y)  (K_ACTMASK_PCT).
  - Dest nodes balance-permuted, sharded 8 x (98 blocks x 128); weights
    replicated; per-layer tables (dinv-prescaled transformed features)
    produced shard-wise and AllGathered; 'own' runs gather from the local
    shard table to hide AllGather latency.
"""

import sys
import types

if "/opt/trn_rl_repo" not in sys.path:
    sys.path.insert(0, "/opt/trn_rl_repo")

import numpy as np


def _install_ntff_shim():
    """antenv.axon_hooks is missing in this container; provide it so
    trace=True (NTFF profiling) works under axon."""
    if "antenv.axon_hooks" in sys.modules:
        return
    try:
        import antenv  # noqa: F401
    except ImportError:
        return
    shim = types.ModuleType("antenv.axon_hooks")
    shim._hook = None

    def set_axon_ntff_profile_hook(h):
        shim._hook = h

    def get_axon_ntff_profile_hook():
        return shim._hook

    shim.set_axon_ntff_profile_hook = set_axon_ntff_profile_hook
    shim.get_axon_ntff_profile_hook = get_axon_ntff_profile_hook
    sys.modules["antenv.axon_hooks"] = shim
    import antenv

    antenv.axon_hooks = shim
    try:
        from trn_agent_boot.trn_boot import _ntff_profile_via_ctypes

        shim._hook = _ntff_profile_via_ctypes("/opt/axon/libaxon_pjrt.so")
    except Exception:
        pass


import os

N_CORES = 8
P = 128
NB = 98                     # dest blocks per core
SHARD = NB * P              # 12544
NPAD = N_CORES * SHARD      # 100352
CROWS = NPAD // 4           # 25088 rows per global chunk (int16-safe)
SEPT = int(os.environ.get("K_SEPT", "7"))    # blocks per dest group
# HW: only ONE live matmul accumulation group per PSUM bank (verified by
# micro-test: interleaved start/accum in one bank drops accumulations), so
# every open dest block needs its own bank: SEPT+head <= 8.
assert SEPT <= 7
NSEPT = NB // SEPT
assert NSEPT * SEPT == NB
NSRC = 5                    # source tables: 0=own shard, 1..4=global chunks
CALLMAX = int(os.environ.get("K_CALLMAX", "4096"))   # gather idxs per call
SCRATCH = int(os.environ.get("K_SCRATCH", "16384"))  # SWDGE ring bytes
NQUEUES = int(os.environ.get("K_NQUEUES", "1"))      # SWDGE queues
BF16TAB = int(os.environ.get("K_BF16TAB", "1"))      # bf16 padded tables
ACTPCT = int(os.environ.get("K_ACTMASK_PCT", "33"))  # % masks on Act engine
TPAD = 128                  # padded table row (bf16 elems) = 256B stride
OWN_FROM_FULL = False       # debug: route own edges via the global table


class Cfg:
    def __init__(self, n_real, f_in, f_out, blocks_per_core, sb_blocks=7):
        assert blocks_per_core == NB
        self.n_real = n_real
        self.f = f_in
        self.f2 = f_out
        self.nb = blocks_per_core
        self.shard = SHARD
        self.npad = NPAD
        assert self.npad >= n_real


def _host_prep(cfg, in_feat, edge_index, edge_weight):
    """Index/layout preprocessing (numpy only; all float math on device
    except copying edge weights into mask-scalar layouts)."""
    n, f = in_feat.shape
    assert n == cfg.n_real and f == cfg.f

    src = np.asarray(edge_index[0], dtype=np.int64)
    dst = np.asarray(edge_index[1], dtype=np.int64)
    ew = np.asarray(edge_weight, dtype=np.float32)
    loop = np.arange(n, dtype=np.int64)      # self loops, weight 1
    src = np.concatenate([src, loop])
    dst = np.concatenate([dst, loop])
    ew = np.concatenate([ew, np.ones(n, np.float32)])

    # ---- balance-permute dest nodes into blocks of 128 slots -----------
    cnt = np.bincount(dst, minlength=NPAD)
    order = np.argsort(-cnt, kind="stable")
    pos = np.arange(NPAD, dtype=np.int64)
    nblocks = N_CORES * NB
    pas = pos // nblocks
    bin_idx = pos % nblocks
    odd = (pas % 2) == 1
    bin_idx[odd] = nblocks - 1 - bin_idx[odd]
    nid = np.empty(NPAD, dtype=np.int64)
    nid[order] = bin_idx * P + pas

    dst_n = nid[dst]
    src_n = nid[src]
    L = int(cnt.max())

    core_e = dst_n // SHARD
    b_e = (dst_n % SHARD) // P
    q_e = b_e // SEPT                        # dest group
    src_core = src_n // SHARD
    own = src_core == core_e
    if OWN_FROM_FULL:
        own[:] = False
    s_e = np.where(own, 0, src_n // CROWS + 1).astype(np.int64)
    tidx_e = np.where(own, src_n - core_e * SHARD, src_n % CROWS)

    # ---- max-over-cores segment sizes ----------------------------------
    cnt_bs = np.zeros((N_CORES, NB, NSRC), np.int64)
    np.add.at(cnt_bs, (core_e, b_e, s_e), 1)
    R = cnt_bs.max(axis=0)                   # [NB, NSRC]

    # ---- run / segment layout ------------------------------------------
    seg_off = np.zeros((NB, NSRC), np.int64)   # slot offset within run
    run_base_qs = np.zeros((NSEPT, NSRC), np.int64)
    run_meta = []                              # per ri: (q, s, base, nslots, calls)
    slot_base = 0
    for q in range(NSEPT):
        for s in range(NSRC):
            off = 0
            for b in range(q * SEPT, (q + 1) * SEPT):
                seg_off[b, s] = off
                off += R[b, s]
            nsl = -(-off // P) * P
            calls = []
            cb = 0
            while cb < nsl:
                nn = min(CALLMAX, nsl - cb)
                calls.append((slot_base + cb, nn))
                cb += nn
            run_base_qs[q, s] = slot_base
            run_meta.append((q, s, slot_base, nsl, calls))
            slot_base += nsl
    TOT = slot_base
    assert TOT % 16 == 0

    # ---- events: one mask+matmul per (run, 128-slot group, block) ------
    # Enumerated segment-major so per-edge event ids are arithmetic.
    first_eid = np.full((NB, NSRC), -1, np.int64)
    seg_g0 = np.zeros((NB, NSRC), np.int64)
    ev_ri = []
    ev_g = []
    ev_b = []
    for q in range(NSEPT):
        for s in range(NSRC):
            ri = q * NSRC + s
            for b in range(q * SEPT, (q + 1) * SEPT):
                if R[b, s] == 0:
                    continue
                g0 = seg_off[b, s] // P
                g1 = (seg_off[b, s] + R[b, s] - 1) // P
                first_eid[b, s] = len(ev_ri)
                seg_g0[b, s] = g0
                for g in range(g0, g1 + 1):
                    ev_ri.append(ri)
                    ev_g.append(g)
                    ev_b.append(b)
    NEV = len(ev_ri)
    ev_ri = np.array(ev_ri)
    ev_g = np.array(ev_g)
    ev_b = np.array(ev_b)

    ev_start = np.zeros(NEV, bool)
    ev_stop = np.zeros(NEV, bool)
    for b in range(NB):
        ss = [s for s in range(NSRC) if R[b, s] > 0]
        assert ss, f"block {b} has no edges?"
        ev_start[first_eid[b, ss[0]]] = True
        ls = ss[-1]
        g0 = seg_off[b, ls] // P
        g1 = (seg_off[b, ls] + R[b, ls] - 1) // P
        ev_stop[first_eid[b, ls] + (g1 - g0)] = True

    # program iteration order: by (run, group, block)
    evprog = np.lexsort((ev_b, ev_g, ev_ri))

    # ---- per-edge slot / event / partition -----------------------------
    segkey = (core_e * NB + b_e) * NSRC + s_e
    eorder = np.argsort(segkey, kind="stable")
    key_s = segkey[eorder]
    seg_start = np.searchsorted(key_s, np.arange(N_CORES * NB * NSRC))
    rank = np.arange(len(key_s)) - seg_start[key_s]
    core_s = core_e[eorder]
    b_s = b_e[eorder]
    s_s = s_e[eorder]
    q_s = q_e[eorder]
    dst_s = dst_n[eorder]
    ew_s = ew[eorder]
    tidx_s = tidx_e[eorder]

    slotrel = seg_off[b_s, s_s] + rank          # slot within run
    slot = run_base_qs[q_s, s_s] + slotrel      # global slot
    gi_run = slotrel // P
    p_slot = slotrel % P
    eid = first_eid[b_s, s_s] + (gi_run - seg_g0[b_s, s_s])

    colrel_t = np.zeros((N_CORES, P, NEV), np.float32)
    wsel_t = np.zeros((N_CORES, P, NEV), np.float32)
    colrel_t[core_s, p_slot, eid] = (dst_s % P).astype(np.float32)
    wsel_t[core_s, p_slot, eid] = ew_s

    # ---- int16 gather indices (16-partition wrap, replicated to 128) ---
    idx_cols = TOT // 16
    idx16 = np.zeros((N_CORES, 16, idx_cols), np.int16)
    idx16[core_s, slot % 16, slot // 16] = tidx_s.astype(np.int16)
    idx16 = np.broadcast_to(
        idx16[:, None, :, :], (N_CORES, 8, 16, idx_cols)
    ).reshape(N_CORES, P, idx_cols)

    # ---- deg layout (device computes deg = row-sum, dinv = 1/sqrt) -----
    dorder = np.argsort(dst_n, kind="stable")
    dst_d = dst_n[dorder]
    ew_d = ew[dorder]
    dfirst = np.searchsorted(dst_d, np.arange(NPAD))
    rankd = np.arange(len(dst_d)) - dfirst[dst_d]
    assert rankd.max() < L
    ewdeg = np.zeros((N_CORES, P, NB * L), np.float32)
    ewdeg[dst_d // SHARD, dst_d % P, ((dst_d % SHARD) // P) * L + rankd] = ew_d
    zdeg = np.where(cnt == 0)[0]
    if len(zdeg) > 0:
        zn = nid[zdeg]
        ewdeg[zn // SHARD, zn % P, ((zn % SHARD) // P) * L] = 1.0

    # ---- permuted, padded, transposed features -------------------------
    xperm = np.zeros((NPAD, f), np.float32)
    xperm[nid[:n]] = np.asarray(in_feat, np.float32)
    xt_shards = [
        np.ascontiguousarray(xperm[c * SHARD:(c + 1) * SHARD].T)
        for c in range(N_CORES)
    ]

    return dict(
        L=L, nid=nid, xt_shards=xt_shards, NEV=NEV, TOT=TOT,
        colrel_t=colrel_t, wsel_t=wsel_t, ewdeg=ewdeg, idx16=idx16,
        idx_cols=idx_cols, run_meta=run_meta, evprog=evprog,
        ev_ri=ev_ri, ev_g=ev_g, ev_b=ev_b, ev_start=ev_start,
        ev_stop=ev_stop,
    )


def _emit_gather(nc, out_ap, in_ap, idxs_ap, num_idxs, elem_size,
                 elem_step, queue_num=0):
    """Emit InstDMAGatherAnt with elem_size_bytes that need not be a
    multiple of 256 (that bass-level assert is a transpose-mode
    restriction; the HW constraint is stride_bytes%256==0, which holds
    via elem_step). Mirrors BassGpSimd.dma_gather's lowering."""
    from concourse import mybir
    from concourse.ap_utils import ap_is_contiguous

    eng = nc.gpsimd
    assert idxs_ap.dtype == mybir.dt.int16
    assert in_ap.dtype == out_ap.dtype
    assert ap_is_contiguous(in_ap.ap[1:])
    assert ap_is_contiguous(out_ap.ap[1:])
    assert ap_is_contiguous(idxs_ap.ap[1:])
    assert in_ap.ap[-1][1] == out_ap.ap[-1][1] == elem_size
    assert out_ap.ap[0][1] * out_ap.ap[1][1] == -(-num_idxs // 128) * 128
    assert in_ap.ap[0][0] == elem_step
    stride_bytes = elem_step * mybir.dt.size(in_ap.dtype)
    assert stride_bytes % 256 == 0 and stride_bytes // 256 < 256
    _in_ap = eng.lower_ap_dma(in_ap, for_custom_bir_dma=True)
    _idxs_ap = eng.lower_ap(idxs_ap)
    _out_ap = eng.lower_ap(out_ap)
    return eng.add_instruction(
        mybir.InstDMAGatherAnt(
            name=nc.get_next_instruction_name(),
            ins=[
                *_in_ap,
                _idxs_ap,
                eng.lower_val_access(eng.to_reg(num_idxs)),
            ],
            outs=[_out_ap],
            transpose=False,
            num_idxs=num_idxs,
            elem_size=elem_size,
            stride_bytes_256=stride_bytes // 256,
            gen_mode=0,
            single_packet=False,
            queue_num=queue_num,
            sbuf_tokens_per_rank=0,
            sbuf_free_dim_per_rank=0,
            sbuf_free_dim_pad_per_rank=0,
            sbuf_byte_offset=0,
        )
    )


def _build_program(cfg, prep):
    from concourse import bacc, mybir, tile

    f, f2 = cfg.f, cfg.f2
    L, NEV, idx_cols = prep["L"], prep["NEV"], prep["idx_cols"]
    run_meta = prep["run_meta"]
    evprog = prep["evprog"]
    ev_ri, ev_g, ev_b = prep["ev_ri"], prep["ev_g"], prep["ev_b"]
    ev_start, ev_stop = prep["ev_start"], prep["ev_stop"]
    fp32 = mybir.dt.float32
    bf16 = mybir.dt.bfloat16
    Alu = mybir.AluOpType
    Act = mybir.ActivationFunctionType
    tdt = bf16 if BF16TAB else fp32
    trow = TPAD if BF16TAB else f

    # events grouped per (run, call)
    ev_by_call = {}
    for e in evprog:
        ri = int(ev_ri[e])
        g = int(ev_g[e])
        k = (g * P) // CALLMAX
        gc = g - k * (CALLMAX // P)
        ev_by_call.setdefault((ri, k), []).append(
            (gc, int(ev_b[e]), int(e), bool(ev_start[e]), bool(ev_stop[e]))
        )

    nc = bacc.Bacc("TRN2", target_bir_lowering=False, debug=False,
                   num_devices=N_CORES, dynamic_dma_scratch_size=SCRATCH,
                   num_swdge_queues=NQUEUES)

    xt_in = nc.dram_tensor("xt", [f, SHARD], fp32, kind="ExternalInput")
    w1_in = nc.dram_tensor("w1", [f, f], fp32, kind="ExternalInput")
    w2_in = nc.dram_tensor("w2", [f, f2], fp32, kind="ExternalInput")
    b1_in = nc.dram_tensor("b1r", [P, f], fp32, kind="ExternalInput")
    b2_in = nc.dram_tensor("b2r", [P, f2], fp32, kind="ExternalInput")
    idx_in = nc.dram_tensor("idx", [P, idx_cols], mybir.dt.int16,
                            kind="ExternalInput")
    colrel_in = nc.dram_tensor("colrel", [P, NEV], fp32, kind="ExternalInput")
    wsel_in = nc.dram_tensor("wsel", [P, NEV], fp32, kind="ExternalInput")
    ewdeg_in = nc.dram_tensor("ewdeg", [P, NB * L], fp32,
                              kind="ExternalInput")
    out_t = nc.dram_tensor("out", [SHARD, f2], fp32, kind="ExternalOutput")

    xw1_shard = nc.dram_tensor("xw1_shard", [SHARD, trow], tdt,
                               kind="Internal")
    xw1_full = nc.dram_tensor("xw1_full", [NPAD, trow], tdt, kind="Internal",
                              addr_space="Shared")
    h2_shard = nc.dram_tensor("h2_shard", [SHARD, trow], tdt, kind="Internal")
    h2_full = nc.dram_tensor("h2_full", [NPAD, trow], tdt, kind="Internal",
                             addr_space="Shared")

    rg = [list(range(N_CORES))]
    GW = CALLMAX // P          # max groups per call

    with tile.TileContext(nc) as tc:
        with tc.tile_pool(name="const", bufs=1) as cpool:
            # ---- constants ---------------------------------------------
            iota_i = cpool.tile([P, P], mybir.dt.int32, name="iota_i")
            nc.gpsimd.iota(iota_i[:], pattern=[[1, P]], base=0,
                           channel_multiplier=0)
            iota_b = cpool.tile([P, P], bf16, name="iota_b")
            nc.vector.tensor_copy(out=iota_b[:], in_=iota_i[:])
            w1_sb = cpool.tile([f, f], fp32, name="w1_sb")
            nc.sync.dma_start(out=w1_sb[:], in_=w1_in[:])
            w1_bf = cpool.tile([f, f], bf16, name="w1_bf")
            nc.vector.tensor_copy(out=w1_bf[:], in_=w1_sb[:])
            w2_sb = cpool.tile([f, f2], fp32, name="w2_sb")
            nc.sync.dma_start(out=w2_sb[:], in_=w2_in[:])
            w2_bf = cpool.tile([f, f2], bf16, name="w2_bf")
            nc.vector.tensor_copy(out=w2_bf[:], in_=w2_sb[:])
            b1_sb = cpool.tile([P, f], fp32, name="b1_sb")
            nc.sync.dma_start(out=b1_sb[:], in_=b1_in[:])
            b2_sb = cpool.tile([P, f2], fp32, name="b2_sb")
            nc.sync.dma_start(out=b2_sb[:], in_=b2_in[:])
            idx_sb = cpool.tile([P, idx_cols], mybir.dt.int16, name="idx_sb")
            nc.sync.dma_start(out=idx_sb[:], in_=idx_in[:])
            colrel_sb = cpool.tile([P, NEV], fp32, name="colrel_sb")
            nc.sync.dma_start(out=colrel_sb[:], in_=colrel_in[:])
            wsel_sb = cpool.tile([P, NEV], fp32, name="wsel_sb")
            nc.sync.dma_start(out=wsel_sb[:], in_=wsel_in[:])
            # negated copies for Act-engine masks: y=Abs(iota-c);
            # mask=Relu(negw*y + w)
            ncol_sb = cpool.tile([P, NEV], fp32, name="ncol_sb")
            nc.vector.tensor_scalar(out=ncol_sb[:], in0=colrel_sb[:],
                                    scalar1=-1.0, scalar2=None, op0=Alu.mult)
            nwsel_sb = cpool.tile([P, NEV], fp32, name="nwsel_sb")
            nc.vector.tensor_scalar(out=nwsel_sb[:], in0=wsel_sb[:],
                                    scalar1=-1.0, scalar2=None, op0=Alu.mult)
            dinv_sb = cpool.tile([P, NB], fp32, name="dinv_sb")
            out_stage = cpool.tile([P, NB * f2], fp32, name="out_stage")

            # ---- deg -> dinv (core-local) ------------------------------
            with tc.tile_pool(name="deg", bufs=1) as degp:
                ewdeg_sb = degp.tile([P, NB * L], fp32)
                nc.sync.dma_start(out=ewdeg_sb[:], in_=ewdeg_in[:])
                deg_sb = degp.tile([P, NB], fp32)
                for b in range(NB):
                    nc.vector.reduce_sum(
                        out=deg_sb[:, b:b + 1],
                        in_=ewdeg_sb[:, b * L:(b + 1) * L],
                        axis=mybir.AxisListType.X)
                sq_sb = degp.tile([P, NB], fp32)
                nc.scalar.activation(out=sq_sb[:], in_=deg_sb[:],
                                     func=Act.Sqrt)
                nc.vector.reciprocal(out=dinv_sb[:], in_=sq_sb[:])

            # ---- dense: table1 = dinv * (X @ W1) for my shard ----------
            SCH = 14
            with tc.tile_pool(name="xt", bufs=1) as xtp, \
                 tc.tile_pool(name="dps", bufs=4, space="PSUM") as dpp, \
                 tc.tile_pool(name="dst", bufs=2) as dstp:
                xtf = xtp.tile([f, SHARD], fp32, name="xtf")
                nc.sync.dma_start(out=xtf[:], in_=xt_in[:])
                xtb = xtp.tile([f, SHARD], bf16, name="xtb")
                nc.scalar.activation(out=xtb[:], in_=xtf[:], func=Act.Copy)
                for t0 in range(0, NB, SCH):
                    stg = dstp.tile([P, SCH * f], tdt, tag="dstg",
                                    name="dstg")
                    for j in range(SCH):
                        b = t0 + j
                        ps = dpp.tile([P, f], fp32, tag="dps", name="dps")
                        nc.tensor.matmul(out=ps[:],
                                         lhsT=xtb[:, b * P:(b + 1) * P],
                                         rhs=w1_bf[:], start=True, stop=True)
                        nc.vector.tensor_scalar(
                            out=stg[:, j * f:(j + 1) * f], in0=ps[:],
                            scalar1=dinv_sb[:, b:b + 1], scalar2=None,
                            op0=Alu.mult)
                    dst_ap = xw1_shard[t0 * P:(t0 + SCH) * P, :f].rearrange(
                        "(i p) f -> p i f", p=P)
                    nc.sync.dma_start(out=dst_ap, in_=stg[:])

            nc.gpsimd.collective_compute(
                "AllGather", Alu.bypass, replica_groups=rg,
                ins=[xw1_shard[:]], outs=[xw1_full[:]])

            # ---- aggregation layers ------------------------------------
            with tc.tile_pool(name="gst", bufs=3) as gpool, \
                 tc.tile_pool(name="gbf", bufs=3) as bpool, \
                 tc.tile_pool(name="mask", bufs=10) as mpool, \
                 tc.tile_pool(name="work", bufs=4) as wpool:

                def agg_layer(layer, shard_t, full_t):
                    # One PSUM bank per open dest block (HW allows only one
                    # live accumulation group per bank), <=7 open + head.
                    pss = {}                  # b -> psum tile
                    callno = [0]
                    evno = [0]

                    def finish_block(b):
                        ps = pss.pop(b)
                        if layer == 1:
                            pblk = ps[:]
                            t1 = wpool.tile([P, f], fp32, tag="t1",
                                            name="t1")
                            nc.vector.tensor_scalar(
                                out=t1[:], in0=pblk,
                                scalar1=dinv_sb[:, b:b + 1], scalar2=None,
                                op0=Alu.mult)
                            t2 = wpool.tile([P, f], fp32, tag="t2",
                                            name="t2")
                            nc.vector.tensor_tensor(
                                out=t2[:], in0=t1[:], in1=b1_sb[:],
                                op=Alu.add)
                            h2w = wpool.tile([P, f], tdt, tag="h2w",
                                             name="h2w")
                            # dinv>0 so relu(dinv*x) == dinv*relu(x)
                            nc.scalar.activation(
                                out=h2w[:], in_=t2[:], func=Act.Relu,
                                scale=dinv_sb[:, b:b + 1])
                            nc.sync.dma_start(
                                out=h2_shard[b * P:(b + 1) * P, :f],
                                in_=h2w[:])
                        else:
                            pblk = ps[:]
                            lh = wpool.tile([f, P], bf16, tag="lh",
                                            name="lh")
                            nc.vector.tensor_copy(out=lh[:], in_=pblk)
                            ps2 = pph.tile([P, f2], fp32, tag="head",
                                           name="ps2")
                            nc.tensor.matmul(out=ps2[:], lhsT=lh[:],
                                             rhs=w2_bf[:], start=True,
                                             stop=True)
                            t3 = wpool.tile([P, f2], fp32, tag="t3",
                                            name="t3")
                            nc.vector.tensor_scalar(
                                out=t3[:], in0=ps2[:],
                                scalar1=dinv_sb[:, b:b + 1], scalar2=None,
                                op0=Alu.mult)
                            nc.vector.tensor_tensor(
                                out=out_stage[:, b * f2:(b + 1) * f2],
                                in0=t3[:], in1=b2_sb[:], op=Alu.add)

                    def build_mask(e):
                        mask = mpool.tile([P, P], bf16, tag="mask",
                                          name="mask")
                        evno[0] += 1
                        if (evno[0] * ACTPCT) // 100 != \
                           ((evno[0] - 1) * ACTPCT) // 100:
                            # Activation-engine build (2 ops)
                            ytmp = mpool.tile([P, P], bf16, tag="ytmp",
                                              name="ytmp")
                            nc.scalar.activation(
                                out=ytmp[:], in_=iota_b[:], func=Act.Abs,
                                bias=ncol_sb[:, e:e + 1])
                            nc.scalar.activation(
                                out=mask[:], in_=ytmp[:], func=Act.Relu,
                                scale=nwsel_sb[:, e:e + 1],
                                bias=wsel_sb[:, e:e + 1])
                        else:
                            nc.vector.tensor_scalar(
                                out=mask[:], in0=iota_b[:],
                                scalar1=colrel_sb[:, e:e + 1],
                                scalar2=wsel_sb[:, e:e + 1],
                                op0=Alu.is_equal, op1=Alu.mult)
                        return mask

                    for ri, (q, s, base, nsl, calls) in enumerate(run_meta):
                        if s == 0:
                            table = shard_t[:, :f]
                        else:
                            table = full_t[(s - 1) * CROWS:s * CROWS, :f]
                        for k, (cbase, nn) in enumerate(calls):
                            gt = gpool.tile([P, GW * f], tdt, tag="gst",
                                            name=f"gt{layer}_{ri}_{k}")
                            if BF16TAB:
                                _emit_gather(
                                    nc,
                                    gt[:, :nn // P * f].rearrange(
                                        "p (a q) -> p a q", q=f),
                                    table,
                                    idx_sb[:, cbase // 16:(cbase + nn) // 16],
                                    nn, f, TPAD,
                                    queue_num=callno[0] % NQUEUES)
                            else:
                                nc.gpsimd.dma_gather(
                                    gt[:, :nn // P * f].rearrange(
                                        "p (a q) -> p a q", q=f),
                                    table,
                                    idx_sb[:, cbase // 16:(cbase + nn) // 16],
                                    nn, nn, f,
                                    single_packet=False,
                                    queue_num=callno[0] % NQUEUES)
                            callno[0] += 1
                            if BF16TAB:
                                gb_t = gt
                            else:
                                gb_t = bpool.tile([P, GW * f], bf16,
                                                  tag="gbf",
                                                  name=f"gb{layer}_{ri}_{k}")
                                nc.scalar.activation(
                                    out=gb_t[:, :nn // P * f],
                                    in_=gt[:, :nn // P * f], func=Act.Copy)
                            for gc, b, e, st, sp in ev_by_call.get(
                                    (ri, k), []):
                                mask = build_mask(e)
                                msg = gb_t[:, gc * f:(gc + 1) * f]
                                if b not in pss:
                                    if layer == 1:
                                        pss[b] = pp.tile([P, f], fp32,
                                                         tag="agg",
                                                         name="aps")
                                    else:
                                        pss[b] = pp.tile([f, P], fp32,
                                                         tag="agg",
                                                         name="apsT")
                                    st = True
                                if layer == 1:
                                    nc.tensor.matmul(out=pss[b][:],
                                                     lhsT=mask[:], rhs=msg,
                                                     start=st, stop=sp)
                                else:
                                    nc.tensor.matmul(out=pss[b][:],
                                                     lhsT=msg, rhs=mask[:],
                                                     start=st, stop=sp)
                                if sp:
                                    finish_block(b)
                    assert not pss, list(pss)

                with tc.tile_pool(name="agg1", bufs=SEPT,
                                  space="PSUM") as pp:
                    agg_layer(1, xw1_shard, xw1_full)

                nc.gpsimd.collective_compute(
                    "AllGather", Alu.bypass, replica_groups=rg,
                    ins=[h2_shard[:]], outs=[h2_full[:]])

                with tc.tile_pool(name="agg2", bufs=SEPT,
                                  space="PSUM") as pp, \
                     tc.tile_pool(name="head", bufs=1, space="PSUM") as pph:
                    agg_layer(2, h2_shard, h2_full)

            out_ap = out_t[:].rearrange("(b p) f -> p b f", p=P)
            nc.sync.dma_start(out=out_ap, in_=out_stage[:])

    nc.compile()
    return nc


def _make_in_maps(cfg, prep, W1, b1, W2, b2):
    b1r = np.broadcast_to(np.asarray(b1, np.float32), (P, cfg.f)).copy()
    b2r = np.broadcast_to(np.asarray(b2, np.float32), (P, cfg.f2)).copy()
    w1 = np.asarray(W1, np.float32)
    w2 = np.asarray(W2, np.float32)
    in_maps = []
    for c in range(N_CORES):
        in_maps.append({
            "xt": prep["xt_shards"][c],
            "w1": w1, "w2": w2, "b1r": b1r, "b2r": b2r,
            "idx": np.ascontiguousarray(prep["idx16"][c]),
            "colrel": np.ascontiguousarray(prep["colrel_t"][c]),
            "wsel": np.ascontiguousarray(prep["wsel_t"][c]),
            "ewdeg": np.ascontiguousarray(prep["ewdeg"][c]),
        })
    return in_maps


def run(cfg, in_feat, edge_index, edge_weight, W1, b1, W2, b2,
        trace=False, use_sim=False):
    """Returns (output [n_real, f2], BassKernelResults|None)."""
    _install_ntff_shim()
    from concourse import bass_utils

    prep = _host_prep(cfg, in_feat, edge_index, edge_weight)
    nc = _build_program(cfg, prep)
    in_maps = _make_in_maps(cfg, prep, W1, b1, W2, b2)

    if use_sim:
        from concourse.bass_interp import MultiCoreSim
        sim = MultiCoreSim(nc, num_cores=N_CORES)
        for c, (cid, core) in enumerate(sim.cores.items()):
            for k, v in in_maps[c].items():
                core.tensor(k)[:] = v
        sim.simulate()
        shards = [sim.cores[c].tensor("out").copy() for c in sim.cores]
        res = None
    else:
        res = bass_utils.run_bass_kernel_spmd(
            nc, in_maps, core_ids=list(range(N_CORES)), trace=trace)
        shards = [res.results[c]["out"] for c in range(N_CORES)]

    out_perm = np.concatenate(shards, axis=0)  # [npad, f2]
    out = out_perm[prep["nid"][:cfg.n_real]]
    return out, res


def kernel(in_feat, edge_index, edge_weight, W1, b1, W2, b2):
    cfg = Cfg(n_real=100000, f_in=64, f_out=16, blocks_per_core=98)
    out, _ = run(cfg, in_feat, edge_index, edge_weight, W1, b1, W2, b2)
    return np.ascontiguousarray(out.astype(np.float32))


# revision 19
# speedup vs baseline: 1.0534x; 1.0534x over previous
"""Trainium2 Bass kernel for a 2-layer GCN (gnn_message_passing) — v3.

Strategy (8 NeuronCores, SPMD). Profile-driven rebuild of v2:
  - The v2 bottleneck was GPSIMD: dma_gather costs ~5-8ns/index + ~3us
    fixed of Pool-engine time (Q7 descriptor-gen), so v3 uses far fewer,
    far larger gather calls: K_CALLMAX=4096 (one call per run instead of
    ~8) -> 70 calls/layer instead of 266. Calls >1024 indices require
    single_packet=False: with single_packet=True the whole per-engine
    descriptor stream coalesces into ONE DMA packet, and 1024 idx x 256B
    / 16 engines = 16KB is the packet-size ceiling (bigger crashes).
  - PSUM: one bank per open dest block (HW allows only ONE live matmul
    accumulation group per bank — interleaving two groups in one bank
    drops accumulations; verified by micro-test), so SEPT=7 + head.
  - Tables stored bf16 padded to 256B rows ([N,128] bf16, features in
    cols 0..63). dma_gather descriptors then move 128B/row instead of
    256B (elem_size=64, elem_step=128 -> stride 256B satisfies the HW
    stride%256 constraint; the bass-level elem_size_bytes%256 assert is a
    transpose-mode restriction, bypassed by emitting InstDMAGatherAnt
    directly). Halves gather HBM traffic and kills the fp32->bf16 cast.
  - Scatter masks (weighted one-hot [slot x dest] bf16) are built on DVE
    (tensor_scalar is_eq*mult) and optionally a fraction on the idle
    Activation engine via two activations: y=Abs(iota-c) then
    mask=Relu(w - w*y)  (K_ACTMASK_PCT).
  - Dest nodes balance-permuted, sharded 8 x (98 blocks x 128); weights
    replicated; per-layer tables (dinv-prescaled transformed features)
    produced shard-wise and AllGathered; 'own' runs gather from the local
    shard table to hide AllGather latency.
"""

import sys
import types

if "/opt/trn_rl_repo" not in sys.path:
    sys.path.insert(0, "/opt/trn_rl_repo")

import numpy as np


def _install_ntff_shim():
    """antenv.axon_hooks is missing in this container; provide it so
    trace=True (NTFF profiling) works under axon."""
    if "antenv.axon_hooks" in sys.modules:
        return
    try:
        import antenv  # noqa: F401
    except ImportError:
        return
    shim = types.ModuleType("antenv.axon_hooks")
    shim._hook = None

    def set_axon_ntff_profile_hook(h):
        shim._hook = h

    def get_axon_ntff_profile_hook():
        return shim._hook

    shim.set_axon_ntff_profile_hook = set_axon_ntff_profile_hook
    shim.get_axon_ntff_profile_hook = get_axon_ntff_profile_hook
    sys.modules["antenv.axon_hooks"] = shim
    import antenv

    antenv.axon_hooks = shim
    try:
        from trn_agent_boot.trn_boot import _ntff_profile_via_ctypes

        shim._hook = _ntff_profile_via_ctypes("/opt/axon/libaxon_pjrt.so")
    except Exception:
        pass


import os

N_CORES = 8
P = 128
NB = 98                     # dest blocks per core
SHARD = NB * P              # 12544
NPAD = N_CORES * SHARD      # 100352
CROWS = NPAD // 4           # 25088 rows per global chunk (int16-safe)
SEPT = int(os.environ.get("K_SEPT", "7"))    # blocks per dest group
# HW: only ONE live matmul accumulation group per PSUM bank (verified by
# micro-test: interleaved start/accum in one bank drops accumulations), so
# every open dest block needs its own bank: SEPT+head <= 8.
assert SEPT <= 7
NSEPT = NB // SEPT
assert NSEPT * SEPT == NB
NSRC = 5                    # source tables: 0=own shard, 1..4=global chunks
CALLMAX = int(os.environ.get("K_CALLMAX", "4096"))   # gather idxs per call
SCRATCH = int(os.environ.get("K_SCRATCH", "16384"))  # SWDGE ring bytes
NQUEUES = int(os.environ.get("K_NQUEUES", "1"))      # SWDGE queues
BF16TAB = int(os.environ.get("K_BF16TAB", "1"))      # bf16 padded tables
ACTPCT = int(os.environ.get("K_ACTMASK_PCT", "33"))  # % masks on Act engine
TPAD = 128                  # padded table row (bf16 elems) = 256B stride
OWN_FROM_FULL = False       # debug: route own edges via the global table


class Cfg:
    def __init__(self, n_real, f_in, f_out, blocks_per_core, sb_blocks=7):
        assert blocks_per_core == NB
        self.n_real = n_real
        self.f = f_in
        self.f2 = f_out
        self.nb = blocks_per_core
        self.shard = SHARD
        self.npad = NPAD
        assert self.npad >= n_real


def _host_prep(cfg, in_feat, edge_index, edge_weight):
    """Index/layout preprocessing (numpy only; all float math on device
    except copying edge weights into mask-scalar layouts)."""
    n, f = in_feat.shape
    assert n == cfg.n_real and f == cfg.f

    src = np.asarray(edge_index[0], dtype=np.int64)
    dst = np.asarray(edge_index[1], dtype=np.int64)
    ew = np.asarray(edge_weight, dtype=np.float32)
    loop = np.arange(n, dtype=np.int64)      # self loops, weight 1
    # Self-loops are NOT gathered per-edge: the self contribution is an
    # identity-mask matmul on the dest block's own table rows (affine DMA)
    # issued in finish_block. They still count for deg.
    dst_deg = np.concatenate([dst, loop])
    ew_deg = np.concatenate([ew, np.ones(n, np.float32)])

    # ---- balance-permute dest nodes into blocks of 128 slots -----------
    cnt = np.bincount(dst_deg, minlength=NPAD)
    order = np.argsort(-cnt, kind="stable")
    pos = np.arange(NPAD, dtype=np.int64)
    nblocks = N_CORES * NB
    pas = pos // nblocks
    bin_idx = pos % nblocks
    odd = (pas % 2) == 1
    bin_idx[odd] = nblocks - 1 - bin_idx[odd]
    nid = np.empty(NPAD, dtype=np.int64)
    nid[order] = bin_idx * P + pas

    dst_n = nid[dst]
    src_n = nid[src]
    dst_ndeg = nid[dst_deg]
    L = int(cnt.max())

    core_e = dst_n // SHARD
    b_e = (dst_n % SHARD) // P
    q_e = b_e // SEPT                        # dest group
    src_core = src_n // SHARD
    own = src_core == core_e
    if OWN_FROM_FULL:
        own[:] = False
    s_e = np.where(own, 0, src_n // CROWS + 1).astype(np.int64)
    tidx_e = np.where(own, src_n - core_e * SHARD, src_n % CROWS)

    # ---- max-over-cores segment sizes ----------------------------------
    cnt_bs = np.zeros((N_CORES, NB, NSRC), np.int64)
    np.add.at(cnt_bs, (core_e, b_e, s_e), 1)
    R = cnt_bs.max(axis=0)                   # [NB, NSRC]

    # ---- run / segment layout ------------------------------------------
    seg_off = np.zeros((NB, NSRC), np.int64)   # slot offset within run
    run_base_qs = np.zeros((NSEPT, NSRC), np.int64)
    run_meta = []                              # per ri: (q, s, base, nslots, calls)
    slot_base = 0
    for q in range(NSEPT):
        for s in range(NSRC):
            off = 0
            for b in range(q * SEPT, (q + 1) * SEPT):
                seg_off[b, s] = off
                off += R[b, s]
            nsl = -(-off // P) * P
            calls = []
            cb = 0
            while cb < nsl:
                nn = min(CALLMAX, nsl - cb)
                calls.append((slot_base + cb, nn))
                cb += nn
            run_base_qs[q, s] = slot_base
            run_meta.append((q, s, slot_base, nsl, calls))
            slot_base += nsl
    TOT = slot_base
    assert TOT % 16 == 0

    # ---- events: one mask+matmul per (run, 128-slot group, block) ------
    # Enumerated segment-major so per-edge event ids are arithmetic.
    first_eid = np.full((NB, NSRC), -1, np.int64)
    seg_g0 = np.zeros((NB, NSRC), np.int64)
    ev_ri = []
    ev_g = []
    ev_b = []
    for q in range(NSEPT):
        for s in range(NSRC):
            ri = q * NSRC + s
            for b in range(q * SEPT, (q + 1) * SEPT):
                if R[b, s] == 0:
                    continue
                g0 = seg_off[b, s] // P
                g1 = (seg_off[b, s] + R[b, s] - 1) // P
                first_eid[b, s] = len(ev_ri)
                seg_g0[b, s] = g0
                for g in range(g0, g1 + 1):
                    ev_ri.append(ri)
                    ev_g.append(g)
                    ev_b.append(b)
    NEV = len(ev_ri)
    ev_ri = np.array(ev_ri)
    ev_g = np.array(ev_g)
    ev_b = np.array(ev_b)

    ev_start = np.zeros(NEV, bool)
    ev_stop = np.zeros(NEV, bool)
    for b in range(NB):
        ss = [s for s in range(NSRC) if R[b, s] > 0]
        assert ss, f"block {b} has no edges?"
        ev_start[first_eid[b, ss[0]]] = True
        ls = ss[-1]
        g0 = seg_off[b, ls] // P
        g1 = (seg_off[b, ls] + R[b, ls] - 1) // P
        ev_stop[first_eid[b, ls] + (g1 - g0)] = True

    # program iteration order: by (run, group, block)
    evprog = np.lexsort((ev_b, ev_g, ev_ri))

    # ---- per-edge slot / event / partition -----------------------------
    segkey = (core_e * NB + b_e) * NSRC + s_e
    eorder = np.argsort(segkey, kind="stable")
    key_s = segkey[eorder]
    seg_start = np.searchsorted(key_s, np.arange(N_CORES * NB * NSRC))
    rank = np.arange(len(key_s)) - seg_start[key_s]
    core_s = core_e[eorder]
    b_s = b_e[eorder]
    s_s = s_e[eorder]
    q_s = q_e[eorder]
    dst_s = dst_n[eorder]
    ew_s = ew[eorder]
    tidx_s = tidx_e[eorder]

    slotrel = seg_off[b_s, s_s] + rank          # slot within run
    slot = run_base_qs[q_s, s_s] + slotrel      # global slot
    gi_run = slotrel // P
    p_slot = slotrel % P
    eid = first_eid[b_s, s_s] + (gi_run - seg_g0[b_s, s_s])

    colrel_t = np.zeros((N_CORES, P, NEV), np.float32)
    wsel_t = np.zeros((N_CORES, P, NEV), np.float32)
    colrel_t[core_s, p_slot, eid] = (dst_s % P).astype(np.float32)
    wsel_t[core_s, p_slot, eid] = ew_s

    # ---- int16 gather indices (16-partition wrap, replicated to 128) ---
    idx_cols = TOT // 16
    idx16 = np.zeros((N_CORES, 16, idx_cols), np.int16)
    idx16[core_s, slot % 16, slot // 16] = tidx_s.astype(np.int16)
    idx16 = np.broadcast_to(
        idx16[:, None, :, :], (N_CORES, 8, 16, idx_cols)
    ).reshape(N_CORES, P, idx_cols)

    # ---- deg layout (device computes deg = row-sum, dinv = 1/sqrt) -----
    dorder = np.argsort(dst_ndeg, kind="stable")
    dst_d = dst_ndeg[dorder]
    ew_d = ew_deg[dorder]
    dfirst = np.searchsorted(dst_d, np.arange(NPAD))
    rankd = np.arange(len(dst_d)) - dfirst[dst_d]
    assert rankd.max() < L
    ewdeg = np.zeros((N_CORES, P, NB * L), np.float32)
    ewdeg[dst_d // SHARD, dst_d % P, ((dst_d % SHARD) // P) * L + rankd] = ew_d
    zdeg = np.where(cnt == 0)[0]
    if len(zdeg) > 0:
        zn = nid[zdeg]
        ewdeg[zn // SHARD, zn % P, ((zn % SHARD) // P) * L] = 1.0

    # ---- permuted, padded, transposed features -------------------------
    xperm = np.zeros((NPAD, f), np.float32)
    xperm[nid[:n]] = np.asarray(in_feat, np.float32)
    xt_shards = [
        np.ascontiguousarray(xperm[c * SHARD:(c + 1) * SHARD].T)
        for c in range(N_CORES)
    ]

    return dict(
        L=L, nid=nid, xt_shards=xt_shards, NEV=NEV, TOT=TOT,
        colrel_t=colrel_t, wsel_t=wsel_t, ewdeg=ewdeg, idx16=idx16,
        idx_cols=idx_cols, run_meta=run_meta, evprog=evprog,
        ev_ri=ev_ri, ev_g=ev_g, ev_b=ev_b, ev_start=ev_start,
        ev_stop=ev_stop,
    )


def _emit_gather(nc, out_ap, in_ap, idxs_ap, num_idxs, elem_size,
                 elem_step, queue_num=0):
    """Emit InstDMAGatherAnt with elem_size_bytes that need not be a
    multiple of 256 (that bass-level assert is a transpose-mode
    restriction; the HW constraint is stride_bytes%256==0, which holds
    via elem_step). Mirrors BassGpSimd.dma_gather's lowering."""
    from concourse import mybir
    from concourse.ap_utils import ap_is_contiguous

    eng = nc.gpsimd
    assert idxs_ap.dtype == mybir.dt.int16
    assert in_ap.dtype == out_ap.dtype
    assert ap_is_contiguous(in_ap.ap[1:])
    assert ap_is_contiguous(out_ap.ap[1:])
    assert ap_is_contiguous(idxs_ap.ap[1:])
    assert in_ap.ap[-1][1] == out_ap.ap[-1][1] == elem_size
    assert out_ap.ap[0][1] * out_ap.ap[1][1] == -(-num_idxs // 128) * 128
    assert in_ap.ap[0][0] == elem_step
    stride_bytes = elem_step * mybir.dt.size(in_ap.dtype)
    assert stride_bytes % 256 == 0 and stride_bytes // 256 < 256
    _in_ap = eng.lower_ap_dma(in_ap, for_custom_bir_dma=True)
    _idxs_ap = eng.lower_ap(idxs_ap)
    _out_ap = eng.lower_ap(out_ap)
    return eng.add_instruction(
        mybir.InstDMAGatherAnt(
            name=nc.get_next_instruction_name(),
            ins=[
                *_in_ap,
                _idxs_ap,
                eng.lower_val_access(eng.to_reg(num_idxs)),
            ],
            outs=[_out_ap],
            transpose=False,
            num_idxs=num_idxs,
            elem_size=elem_size,
            stride_bytes_256=stride_bytes // 256,
            gen_mode=0,
            single_packet=False,
            queue_num=queue_num,
            sbuf_tokens_per_rank=0,
            sbuf_free_dim_per_rank=0,
            sbuf_free_dim_pad_per_rank=0,
            sbuf_byte_offset=0,
        )
    )


def _build_program(cfg, prep):
    from concourse import bacc, mybir, tile

    f, f2 = cfg.f, cfg.f2
    L, NEV, idx_cols = prep["L"], prep["NEV"], prep["idx_cols"]
    run_meta = prep["run_meta"]
    evprog = prep["evprog"]
    ev_ri, ev_g, ev_b = prep["ev_ri"], prep["ev_g"], prep["ev_b"]
    ev_start, ev_stop = prep["ev_start"], prep["ev_stop"]
    fp32 = mybir.dt.float32
    bf16 = mybir.dt.bfloat16
    Alu = mybir.AluOpType
    Act = mybir.ActivationFunctionType
    tdt = bf16 if BF16TAB else fp32
    trow = TPAD if BF16TAB else f

    # events grouped per (run, call)
    ev_by_call = {}
    for e in evprog:
        ri = int(ev_ri[e])
        g = int(ev_g[e])
        k = (g * P) // CALLMAX
        gc = g - k * (CALLMAX // P)
        ev_by_call.setdefault((ri, k), []).append(
            (gc, int(ev_b[e]), int(e), bool(ev_start[e]), bool(ev_stop[e]))
        )

    nc = bacc.Bacc("TRN2", target_bir_lowering=False, debug=False,
                   num_devices=N_CORES, dynamic_dma_scratch_size=SCRATCH,
                   num_swdge_queues=NQUEUES)

    xt_in = nc.dram_tensor("xt", [f, SHARD], fp32, kind="ExternalInput")
    w1_in = nc.dram_tensor("w1", [f, f], fp32, kind="ExternalInput")
    w2_in = nc.dram_tensor("w2", [f, f2], fp32, kind="ExternalInput")
    b1_in = nc.dram_tensor("b1r", [P, f], fp32, kind="ExternalInput")
    b2_in = nc.dram_tensor("b2r", [P, f2], fp32, kind="ExternalInput")
    idx_in = nc.dram_tensor("idx", [P, idx_cols], mybir.dt.int16,
                            kind="ExternalInput")
    colrel_in = nc.dram_tensor("colrel", [P, NEV], fp32, kind="ExternalInput")
    wsel_in = nc.dram_tensor("wsel", [P, NEV], fp32, kind="ExternalInput")
    ewdeg_in = nc.dram_tensor("ewdeg", [P, NB * L], fp32,
                              kind="ExternalInput")
    out_t = nc.dram_tensor("out", [SHARD, f2], fp32, kind="ExternalOutput")

    xw1_shard = nc.dram_tensor("xw1_shard", [SHARD, trow], tdt,
                               kind="Internal")
    xw1_full = nc.dram_tensor("xw1_full", [NPAD, trow], tdt, kind="Internal",
                              addr_space="Shared")
    h2_shard = nc.dram_tensor("h2_shard", [SHARD, trow], tdt, kind="Internal")
    h2_full = nc.dram_tensor("h2_full", [NPAD, trow], tdt, kind="Internal",
                             addr_space="Shared")

    rg = [list(range(N_CORES))]
    GW = CALLMAX // P          # max groups per call

    with tile.TileContext(nc) as tc:
        with tc.tile_pool(name="const", bufs=1) as cpool:
            # ---- constants ---------------------------------------------
            iota_i = cpool.tile([P, P], mybir.dt.int32, name="iota_i")
            nc.gpsimd.iota(iota_i[:], pattern=[[1, P]], base=0,
                           channel_multiplier=0)
            iota_b = cpool.tile([P, P], bf16, name="iota_b")
            nc.vector.tensor_copy(out=iota_b[:], in_=iota_i[:])
            pidx_i = cpool.tile([P, 1], mybir.dt.int32, name="pidx_i")
            nc.gpsimd.iota(pidx_i[:], pattern=[[0, 1]], base=0,
                           channel_multiplier=1)
            pidx_f = cpool.tile([P, 1], fp32, name="pidx_f")
            nc.vector.tensor_copy(out=pidx_f[:], in_=pidx_i[:])
            ident_b = cpool.tile([P, P], bf16, name="ident_b")
            nc.vector.tensor_scalar(out=ident_b[:], in0=iota_b[:],
                                    scalar1=pidx_f[:, 0:1], scalar2=None,
                                    op0=Alu.is_equal)
            w1_sb = cpool.tile([f, f], fp32, name="w1_sb")
            nc.sync.dma_start(out=w1_sb[:], in_=w1_in[:])
            w1_bf = cpool.tile([f, f], bf16, name="w1_bf")
            nc.vector.tensor_copy(out=w1_bf[:], in_=w1_sb[:])
            w2_sb = cpool.tile([f, f2], fp32, name="w2_sb")
            nc.sync.dma_start(out=w2_sb[:], in_=w2_in[:])
            w2_bf = cpool.tile([f, f2], bf16, name="w2_bf")
            nc.vector.tensor_copy(out=w2_bf[:], in_=w2_sb[:])
            b1_sb = cpool.tile([P, f], fp32, name="b1_sb")
            nc.sync.dma_start(out=b1_sb[:], in_=b1_in[:])
            b2_sb = cpool.tile([P, f2], fp32, name="b2_sb")
            nc.sync.dma_start(out=b2_sb[:], in_=b2_in[:])
            idx_sb = cpool.tile([P, idx_cols], mybir.dt.int16, name="idx_sb")
            nc.sync.dma_start(out=idx_sb[:], in_=idx_in[:])
            colrel_sb = cpool.tile([P, NEV], fp32, name="colrel_sb")
            nc.sync.dma_start(out=colrel_sb[:], in_=colrel_in[:])
            wsel_sb = cpool.tile([P, NEV], fp32, name="wsel_sb")
            nc.sync.dma_start(out=wsel_sb[:], in_=wsel_in[:])
            # negated copies for Act-engine masks: y=Abs(iota-c);
            # mask=Relu(negw*y + w)
            ncol_sb = cpool.tile([P, NEV], fp32, name="ncol_sb")
            nc.vector.tensor_scalar(out=ncol_sb[:], in0=colrel_sb[:],
                                    scalar1=-1.0, scalar2=None, op0=Alu.mult)
            nwsel_sb = cpool.tile([P, NEV], fp32, name="nwsel_sb")
            nc.vector.tensor_scalar(out=nwsel_sb[:], in0=wsel_sb[:],
                                    scalar1=-1.0, scalar2=None, op0=Alu.mult)
            dinv_sb = cpool.tile([P, NB], fp32, name="dinv_sb")
            out_stage = cpool.tile([P, NB * f2], fp32, name="out_stage")

            # ---- deg -> dinv (core-local) ------------------------------
            with tc.tile_pool(name="deg", bufs=1) as degp:
                ewdeg_sb = degp.tile([P, NB * L], fp32)
                nc.sync.dma_start(out=ewdeg_sb[:], in_=ewdeg_in[:])
                deg_sb = degp.tile([P, NB], fp32)
                for b in range(NB):
                    nc.vector.reduce_sum(
                        out=deg_sb[:, b:b + 1],
                        in_=ewdeg_sb[:, b * L:(b + 1) * L],
                        axis=mybir.AxisListType.X)
                sq_sb = degp.tile([P, NB], fp32)
                nc.scalar.activation(out=sq_sb[:], in_=deg_sb[:],
                                     func=Act.Sqrt)
                nc.vector.reciprocal(out=dinv_sb[:], in_=sq_sb[:])

            # ---- dense: table1 = dinv * (X @ W1) for my shard ----------
            SCH = 14
            with tc.tile_pool(name="xt", bufs=1) as xtp, \
                 tc.tile_pool(name="dps", bufs=4, space="PSUM") as dpp, \
                 tc.tile_pool(name="dst", bufs=2) as dstp:
                xtf = xtp.tile([f, SHARD], fp32, name="xtf")
                nc.sync.dma_start(out=xtf[:], in_=xt_in[:])
                xtb = xtp.tile([f, SHARD], bf16, name="xtb")
                nc.scalar.activation(out=xtb[:], in_=xtf[:], func=Act.Copy)
                for t0 in range(0, NB, SCH):
                    stg = dstp.tile([P, SCH * f], tdt, tag="dstg",
                                    name="dstg")
                    for j in range(SCH):
                        b = t0 + j
                        ps = dpp.tile([P, f], fp32, tag="dps", name="dps")
                        nc.tensor.matmul(out=ps[:],
                                         lhsT=xtb[:, b * P:(b + 1) * P],
                                         rhs=w1_bf[:], start=True, stop=True)
                        nc.vector.tensor_scalar(
                            out=stg[:, j * f:(j + 1) * f], in0=ps[:],
                            scalar1=dinv_sb[:, b:b + 1], scalar2=None,
                            op0=Alu.mult)
                    dst_ap = xw1_shard[t0 * P:(t0 + SCH) * P, :f].rearrange(
                        "(i p) f -> p i f", p=P)
                    nc.sync.dma_start(out=dst_ap, in_=stg[:])

            nc.gpsimd.collective_compute(
                "AllGather", Alu.bypass, replica_groups=rg,
                ins=[xw1_shard[:]], outs=[xw1_full[:]])

            # ---- aggregation layers ------------------------------------
            with tc.tile_pool(name="gst", bufs=3) as gpool, \
                 tc.tile_pool(name="gbf", bufs=3) as bpool, \
                 tc.tile_pool(name="mask", bufs=10) as mpool, \
                 tc.tile_pool(name="work", bufs=4) as wpool:

                def agg_layer(layer, shard_t, full_t):
                    # One PSUM bank per open dest block (HW allows only one
                    # live accumulation group per bank), <=7 open + head.
                    pss = {}                  # b -> psum tile
                    callno = [0]
                    evno = [0]

                    def finish_block(b):
                        ps = pss.pop(b)
                        # self-loop: identity-mask matmul on this block's
                        # own table rows (affine DMA, no gather slot)
                        srow = wpool.tile([P, f], tdt, tag="srow",
                                          name="srow")
                        nc.sync.dma_start(
                            out=srow[:],
                            in_=shard_t[b * P:(b + 1) * P, :f])
                        if layer == 1:
                            nc.tensor.matmul(out=ps[:], lhsT=ident_b[:],
                                             rhs=srow[:], start=False,
                                             stop=True)
                        else:
                            nc.tensor.matmul(out=ps[:], lhsT=srow[:],
                                             rhs=ident_b[:], start=False,
                                             stop=True)
                        if layer == 1:
                            pblk = ps[:]
                            t1 = wpool.tile([P, f], fp32, tag="t1",
                                            name="t1")
                            nc.vector.tensor_scalar(
                                out=t1[:], in0=pblk,
                                scalar1=dinv_sb[:, b:b + 1], scalar2=None,
                                op0=Alu.mult)
                            t2 = wpool.tile([P, f], fp32, tag="t2",
                                            name="t2")
                            nc.vector.tensor_tensor(
                                out=t2[:], in0=t1[:], in1=b1_sb[:],
                                op=Alu.add)
                            h2w = wpool.tile([P, f], tdt, tag="h2w",
                                             name="h2w")
                            # dinv>0 so relu(dinv*x) == dinv*relu(x)
                            nc.scalar.activation(
                                out=h2w[:], in_=t2[:], func=Act.Relu,
                                scale=dinv_sb[:, b:b + 1])
                            nc.sync.dma_start(
                                out=h2_shard[b * P:(b + 1) * P, :f],
                                in_=h2w[:])
                        else:
                            pblk = ps[:]
                            lh = wpool.tile([f, P], bf16, tag="lh",
                                            name="lh")
                            nc.vector.tensor_copy(out=lh[:], in_=pblk)
                            ps2 = pph.tile([P, f2], fp32, tag="head",
                                           name="ps2")
                            nc.tensor.matmul(out=ps2[:], lhsT=lh[:],
                                             rhs=w2_bf[:], start=True,
                                             stop=True)
                            t3 = wpool.tile([P, f2], fp32, tag="t3",
                                            name="t3")
                            nc.vector.tensor_scalar(
                                out=t3[:], in0=ps2[:],
                                scalar1=dinv_sb[:, b:b + 1], scalar2=None,
                                op0=Alu.mult)
                            nc.vector.tensor_tensor(
                                out=out_stage[:, b * f2:(b + 1) * f2],
                                in0=t3[:], in1=b2_sb[:], op=Alu.add)

                    def build_mask(e):
                        mask = mpool.tile([P, P], bf16, tag="mask",
                                          name="mask")
                        evno[0] += 1
                        if (evno[0] * ACTPCT) // 100 != \
                           ((evno[0] - 1) * ACTPCT) // 100:
                            # Activation-engine build (2 ops)
                            ytmp = mpool.tile([P, P], bf16, tag="ytmp",
                                              name="ytmp")
                            nc.scalar.activation(
                                out=ytmp[:], in_=iota_b[:], func=Act.Abs,
                                bias=ncol_sb[:, e:e + 1])
                            nc.scalar.activation(
                                out=mask[:], in_=ytmp[:], func=Act.Relu,
                                scale=nwsel_sb[:, e:e + 1],
                                bias=wsel_sb[:, e:e + 1])
                        else:
                            nc.vector.tensor_scalar(
                                out=mask[:], in0=iota_b[:],
                                scalar1=colrel_sb[:, e:e + 1],
                                scalar2=wsel_sb[:, e:e + 1],
                                op0=Alu.is_equal, op1=Alu.mult)
                        return mask

                    for ri, (q, s, base, nsl, calls) in enumerate(run_meta):
                        if s == 0:
                            table = shard_t[:, :f]
                        else:
                            table = full_t[(s - 1) * CROWS:s * CROWS, :f]
                        for k, (cbase, nn) in enumerate(calls):
                            gt = gpool.tile([P, GW * f], tdt, tag="gst",
                                            name=f"gt{layer}_{ri}_{k}")
                            if BF16TAB:
                                _emit_gather(
                                    nc,
                                    gt[:, :nn // P * f].rearrange(
                                        "p (a q) -> p a q", q=f),
                                    table,
                                    idx_sb[:, cbase // 16:(cbase + nn) // 16],
                                    nn, f, TPAD,
                                    queue_num=callno[0] % NQUEUES)
                            else:
                                nc.gpsimd.dma_gather(
                                    gt[:, :nn // P * f].rearrange(
                                        "p (a q) -> p a q", q=f),
                                    table,
                                    idx_sb[:, cbase // 16:(cbase + nn) // 16],
                                    nn, nn, f,
                                    single_packet=False,
                                    queue_num=callno[0] % NQUEUES)
                            callno[0] += 1
                            if BF16TAB:
                                gb_t = gt
                            else:
                                gb_t = bpool.tile([P, GW * f], bf16,
                                                  tag="gbf",
                                                  name=f"gb{layer}_{ri}_{k}")
                                nc.scalar.activation(
                                    out=gb_t[:, :nn // P * f],
                                    in_=gt[:, :nn // P * f], func=Act.Copy)
                            for gc, b, e, st, sp in ev_by_call.get(
                                    (ri, k), []):
                                mask = build_mask(e)
                                msg = gb_t[:, gc * f:(gc + 1) * f]
                                if b not in pss:
                                    if layer == 1:
                                        pss[b] = pp.tile([P, f], fp32,
                                                         tag="agg",
                                                         name="aps")
                                    else:
                                        pss[b] = pp.tile([f, P], fp32,
                                                         tag="agg",
                                                         name="apsT")
                                    st = True
                                if layer == 1:
                                    nc.tensor.matmul(out=pss[b][:],
                                                     lhsT=mask[:], rhs=msg,
                                                     start=st, stop=False)
                                else:
                                    nc.tensor.matmul(out=pss[b][:],
                                                     lhsT=msg, rhs=mask[:],
                                                     start=st, stop=False)
                                if sp:
                                    finish_block(b)
                    assert not pss, list(pss)

                with tc.tile_pool(name="agg1", bufs=SEPT,
                                  space="PSUM") as pp:
                    agg_layer(1, xw1_shard, xw1_full)

                nc.gpsimd.collective_compute(
                    "AllGather", Alu.bypass, replica_groups=rg,
                    ins=[h2_shard[:]], outs=[h2_full[:]])

                with tc.tile_pool(name="agg2", bufs=SEPT,
                                  space="PSUM") as pp, \
                     tc.tile_pool(name="head", bufs=1, space="PSUM") as pph:
                    agg_layer(2, h2_shard, h2_full)

            out_ap = out_t[:].rearrange("(b p) f -> p b f", p=P)
            nc.sync.dma_start(out=out_ap, in_=out_stage[:])

    nc.compile()
    return nc


def _make_in_maps(cfg, prep, W1, b1, W2, b2):
    b1r = np.broadcast_to(np.asarray(b1, np.float32), (P, cfg.f)).copy()
    b2r = np.broadcast_to(np.asarray(b2, np.float32), (P, cfg.f2)).copy()
    w1 = np.asarray(W1, np.float32)
    w2 = np.asarray(W2, np.float32)
    in_maps = []
    for c in range(N_CORES):
        in_maps.append({
            "xt": prep["xt_shards"][c],
            "w1": w1, "w2": w2, "b1r": b1r, "b2r": b2r,
            "idx": np.ascontiguousarray(prep["idx16"][c]),
            "colrel": np.ascontiguousarray(prep["colrel_t"][c]),
            "wsel": np.ascontiguousarray(prep["wsel_t"][c]),
            "ewdeg": np.ascontiguousarray(prep["ewdeg"][c]),
        })
    return in_maps


def run(cfg, in_feat, edge_index, edge_weight, W1, b1, W2, b2,
        trace=False, use_sim=False):
    """Returns (output [n_real, f2], BassKernelResults|None)."""
    _install_ntff_shim()
    from concourse import bass_utils

    prep = _host_prep(cfg, in_feat, edge_index, edge_weight)
    nc = _build_program(cfg, prep)
    in_maps = _make_in_maps(cfg, prep, W1, b1, W2, b2)

    if use_sim:
        from concourse.bass_interp import MultiCoreSim
        sim = MultiCoreSim(nc, num_cores=N_CORES)
        for c, (cid, core) in enumerate(sim.cores.items()):
            for k, v in in_maps[c].items():
                core.tensor(k)[:] = v
        sim.simulate()
        shards = [sim.cores[c].tensor("out").copy() for c in sim.cores]
        res = None
    else:
        res = bass_utils.run_bass_kernel_spmd(
            nc, in_maps, core_ids=list(range(N_CORES)), trace=trace)
        shards = [res.results[c]["out"] for c in range(N_CORES)]

    out_perm = np.concatenate(shards, axis=0)  # [npad, f2]
    out = out_perm[prep["nid"][:cfg.n_real]]
    return out, res


def kernel(in_feat, edge_index, edge_weight, W1, b1, W2, b2):
    cfg = Cfg(n_real=100000, f_in=64, f_out=16, blocks_per_core=98)
    out, _ = run(cfg, in_feat, edge_index, edge_weight, W1, b1, W2, b2)
    return np.ascontiguousarray(out.astype(np.float32))


# revision 24
# speedup vs baseline: 1.2627x; 1.1987x over previous
"""Trainium2 Bass kernel for a 2-layer GCN (gnn_message_passing) — v3.

Strategy (8 NeuronCores, SPMD). Profile-driven rebuild of v2:
  - The v2 bottleneck was GPSIMD: dma_gather costs ~5-8ns/index + ~3us
    fixed of Pool-engine time (Q7 descriptor-gen), so v3 uses far fewer,
    far larger gather calls: K_CALLMAX=4096 (one call per run instead of
    ~8) -> 70 calls/layer instead of 266. Calls >1024 indices require
    single_packet=False: with single_packet=True the whole per-engine
    descriptor stream coalesces into ONE DMA packet, and 1024 idx x 256B
    / 16 engines = 16KB is the packet-size ceiling (bigger crashes).
  - PSUM: one bank per open dest block (HW allows only ONE live matmul
    accumulation group per bank — interleaving two groups in one bank
    drops accumulations; verified by micro-test), so SEPT=7 + head.
  - Tables stored bf16 padded to 256B rows ([N,128] bf16, features in
    cols 0..63). dma_gather descriptors then move 128B/row instead of
    256B (elem_size=64, elem_step=128 -> stride 256B satisfies the HW
    stride%256 constraint; the bass-level elem_size_bytes%256 assert is a
    transpose-mode restriction, bypassed by emitting InstDMAGatherAnt
    directly). Halves gather HBM traffic and kills the fp32->bf16 cast.
  - Scatter masks (weighted one-hot [slot x dest] bf16) are built on DVE
    (tensor_scalar is_eq*mult) and optionally a fraction on the idle
    Activation engine via two activations: y=Abs(iota-c) then
    mask=Relu(w - w*y)  (K_ACTMASK_PCT).
  - Dest nodes balance-permuted, sharded 8 x (98 blocks x 128); weights
    replicated; per-layer tables (dinv-prescaled transformed features)
    produced shard-wise and AllGathered; 'own' runs gather from the local
    shard table to hide AllGather latency.
"""

import sys
import types

if "/opt/trn_rl_repo" not in sys.path:
    sys.path.insert(0, "/opt/trn_rl_repo")

import numpy as np


def _install_ntff_shim():
    """antenv.axon_hooks is missing in this container; provide it so
    trace=True (NTFF profiling) works under axon."""
    if "antenv.axon_hooks" in sys.modules:
        return
    try:
        import antenv  # noqa: F401
    except ImportError:
        return
    shim = types.ModuleType("antenv.axon_hooks")
    shim._hook = None

    def set_axon_ntff_profile_hook(h):
        shim._hook = h

    def get_axon_ntff_profile_hook():
        return shim._hook

    shim.set_axon_ntff_profile_hook = set_axon_ntff_profile_hook
    shim.get_axon_ntff_profile_hook = get_axon_ntff_profile_hook
    sys.modules["antenv.axon_hooks"] = shim
    import antenv

    antenv.axon_hooks = shim
    try:
        from trn_agent_boot.trn_boot import _ntff_profile_via_ctypes

        shim._hook = _ntff_profile_via_ctypes("/opt/axon/libaxon_pjrt.so")
    except Exception:
        pass


import os

N_CORES = 8
P = 128
NB = 98                     # dest blocks per core
SHARD = NB * P              # 12544
NPAD = N_CORES * SHARD      # 100352
CROWS = NPAD // 4           # 25088 rows per global chunk (int16-safe)
QROWS = SHARD // 4          # 3136: shard quarter; chunk j is the
                            # rank-concat of all cores' quarter j, so one
                            # quarter-slice AllGather produces chunk j and
                            # global runs only wait for their own chunk
SEPT = int(os.environ.get("K_SEPT", "7"))    # blocks per dest group
# HW: only ONE live matmul accumulation group per PSUM bank (verified by
# micro-test: interleaved start/accum in one bank drops accumulations), so
# every open dest block needs its own bank: SEPT+head <= 8.
assert SEPT <= 7
NSEPT = NB // SEPT
assert NSEPT * SEPT == NB
NSRC = 5                    # source tables: 0=own shard, 1..4=global chunks
CALLMAX = int(os.environ.get("K_CALLMAX", "4096"))   # gather idxs per call
SCRATCH = int(os.environ.get("K_SCRATCH", "16384"))  # SWDGE ring bytes
NQUEUES = int(os.environ.get("K_NQUEUES", "1"))      # SWDGE queues
BF16TAB = int(os.environ.get("K_BF16TAB", "1"))      # bf16 padded tables
ACTPCT = int(os.environ.get("K_ACTMASK_PCT", "33"))  # % masks on Act engine
TPAD = 128                  # padded table row (bf16 elems) = 256B stride
OWN_FROM_FULL = False       # debug: route own edges via the global table


class Cfg:
    def __init__(self, n_real, f_in, f_out, blocks_per_core, sb_blocks=7):
        assert blocks_per_core == NB
        self.n_real = n_real
        self.f = f_in
        self.f2 = f_out
        self.nb = blocks_per_core
        self.shard = SHARD
        self.npad = NPAD
        assert self.npad >= n_real


def _host_prep(cfg, in_feat, edge_index, edge_weight):
    """Index/layout preprocessing (numpy only; all float math on device
    except copying edge weights into mask-scalar layouts)."""
    n, f = in_feat.shape
    assert n == cfg.n_real and f == cfg.f

    src = np.asarray(edge_index[0], dtype=np.int64)
    dst = np.asarray(edge_index[1], dtype=np.int64)
    ew = np.asarray(edge_weight, dtype=np.float32)
    loop = np.arange(n, dtype=np.int64)      # self loops, weight 1
    # Self-loops are NOT gathered per-edge: the self contribution is an
    # identity-mask matmul on the dest block's own table rows (affine DMA)
    # issued in finish_block. They still count for deg.
    dst_deg = np.concatenate([dst, loop])
    ew_deg = np.concatenate([ew, np.ones(n, np.float32)])

    # ---- balance-permute dest nodes into blocks of 128 slots -----------
    cnt = np.bincount(dst_deg, minlength=NPAD)
    order = np.argsort(-cnt, kind="stable")
    pos = np.arange(NPAD, dtype=np.int64)
    nblocks = N_CORES * NB
    pas = pos // nblocks
    bin_idx = pos % nblocks
    odd = (pas % 2) == 1
    bin_idx[odd] = nblocks - 1 - bin_idx[odd]
    nid = np.empty(NPAD, dtype=np.int64)
    nid[order] = bin_idx * P + pas

    dst_n = nid[dst]
    src_n = nid[src]
    dst_ndeg = nid[dst_deg]
    L = int(cnt.max())

    core_e = dst_n // SHARD
    b_e = (dst_n % SHARD) // P
    q_e = b_e // SEPT                        # dest group
    src_core = src_n // SHARD
    own = src_core == core_e
    if OWN_FROM_FULL:
        own[:] = False
    src_r = src_n % SHARD
    s_e = np.where(own, 0, src_r // QROWS + 1).astype(np.int64)
    tidx_e = np.where(own, src_r,
                      (src_n // SHARD) * QROWS + src_r % QROWS)

    # ---- max-over-cores segment sizes ----------------------------------
    cnt_bs = np.zeros((N_CORES, NB, NSRC), np.int64)
    np.add.at(cnt_bs, (core_e, b_e, s_e), 1)
    R = cnt_bs.max(axis=0)                   # [NB, NSRC]

    # ---- run / segment layout ------------------------------------------
    seg_off = np.zeros((NB, NSRC), np.int64)   # slot offset within run
    run_base_qs = np.zeros((NSEPT, NSRC), np.int64)
    run_meta = []                              # per ri: (q, s, base, nslots, calls)
    slot_base = 0
    for q in range(NSEPT):
        for s in range(NSRC):
            off = 0
            for b in range(q * SEPT, (q + 1) * SEPT):
                seg_off[b, s] = off
                off += R[b, s]
            nsl = -(-off // P) * P
            calls = []
            cb = 0
            while cb < nsl:
                nn = min(CALLMAX, nsl - cb)
                calls.append((slot_base + cb, nn))
                cb += nn
            run_base_qs[q, s] = slot_base
            run_meta.append((q, s, slot_base, nsl, calls))
            slot_base += nsl
    TOT = slot_base
    assert TOT % 16 == 0

    # ---- events: one mask+matmul per (run, 128-slot group, block) ------
    # Enumerated segment-major so per-edge event ids are arithmetic.
    first_eid = np.full((NB, NSRC), -1, np.int64)
    seg_g0 = np.zeros((NB, NSRC), np.int64)
    ev_ri = []
    ev_g = []
    ev_b = []
    for q in range(NSEPT):
        for s in range(NSRC):
            ri = q * NSRC + s
            for b in range(q * SEPT, (q + 1) * SEPT):
                if R[b, s] == 0:
                    continue
                g0 = seg_off[b, s] // P
                g1 = (seg_off[b, s] + R[b, s] - 1) // P
                first_eid[b, s] = len(ev_ri)
                seg_g0[b, s] = g0
                for g in range(g0, g1 + 1):
                    ev_ri.append(ri)
                    ev_g.append(g)
                    ev_b.append(b)
    NEV = len(ev_ri)
    ev_ri = np.array(ev_ri)
    ev_g = np.array(ev_g)
    ev_b = np.array(ev_b)

    ev_start = np.zeros(NEV, bool)
    ev_stop = np.zeros(NEV, bool)
    for b in range(NB):
        ss = [s for s in range(NSRC) if R[b, s] > 0]
        assert ss, f"block {b} has no edges?"
        ev_start[first_eid[b, ss[0]]] = True
        ls = ss[-1]
        g0 = seg_off[b, ls] // P
        g1 = (seg_off[b, ls] + R[b, ls] - 1) // P
        ev_stop[first_eid[b, ls] + (g1 - g0)] = True

    # program iteration order: by (run, group, block)
    evprog = np.lexsort((ev_b, ev_g, ev_ri))

    # ---- per-edge slot / event / partition -----------------------------
    segkey = (core_e * NB + b_e) * NSRC + s_e
    eorder = np.argsort(segkey, kind="stable")
    key_s = segkey[eorder]
    seg_start = np.searchsorted(key_s, np.arange(N_CORES * NB * NSRC))
    rank = np.arange(len(key_s)) - seg_start[key_s]
    core_s = core_e[eorder]
    b_s = b_e[eorder]
    s_s = s_e[eorder]
    q_s = q_e[eorder]
    dst_s = dst_n[eorder]
    ew_s = ew[eorder]
    tidx_s = tidx_e[eorder]

    slotrel = seg_off[b_s, s_s] + rank          # slot within run
    slot = run_base_qs[q_s, s_s] + slotrel      # global slot
    gi_run = slotrel // P
    p_slot = slotrel % P
    eid = first_eid[b_s, s_s] + (gi_run - seg_g0[b_s, s_s])

    colrel_t = np.zeros((N_CORES, P, NEV), np.float32)
    wsel_t = np.zeros((N_CORES, P, NEV), np.float32)
    colrel_t[core_s, p_slot, eid] = (dst_s % P).astype(np.float32)
    wsel_t[core_s, p_slot, eid] = ew_s

    # ---- int16 gather indices (16-partition wrap, replicated to 128) ---
    idx_cols = TOT // 16
    idx16 = np.zeros((N_CORES, 16, idx_cols), np.int16)
    idx16[core_s, slot % 16, slot // 16] = tidx_s.astype(np.int16)
    idx16 = np.broadcast_to(
        idx16[:, None, :, :], (N_CORES, 8, 16, idx_cols)
    ).reshape(N_CORES, P, idx_cols)

    # ---- deg layout (device computes deg = row-sum, dinv = 1/sqrt) -----
    dorder = np.argsort(dst_ndeg, kind="stable")
    dst_d = dst_ndeg[dorder]
    ew_d = ew_deg[dorder]
    dfirst = np.searchsorted(dst_d, np.arange(NPAD))
    rankd = np.arange(len(dst_d)) - dfirst[dst_d]
    assert rankd.max() < L
    ewdeg = np.zeros((N_CORES, P, NB * L), np.float32)
    ewdeg[dst_d // SHARD, dst_d % P, ((dst_d % SHARD) // P) * L + rankd] = ew_d
    zdeg = np.where(cnt == 0)[0]
    if len(zdeg) > 0:
        zn = nid[zdeg]
        ewdeg[zn // SHARD, zn % P, ((zn % SHARD) // P) * L] = 1.0

    # ---- permuted, padded, transposed features -------------------------
    xperm = np.zeros((NPAD, f), np.float32)
    xperm[nid[:n]] = np.asarray(in_feat, np.float32)
    xt_shards = [
        np.ascontiguousarray(xperm[c * SHARD:(c + 1) * SHARD].T)
        for c in range(N_CORES)
    ]

    return dict(
        L=L, nid=nid, xt_shards=xt_shards, NEV=NEV, TOT=TOT,
        colrel_t=colrel_t, wsel_t=wsel_t, ewdeg=ewdeg, idx16=idx16,
        idx_cols=idx_cols, run_meta=run_meta, evprog=evprog,
        ev_ri=ev_ri, ev_g=ev_g, ev_b=ev_b, ev_start=ev_start,
        ev_stop=ev_stop,
    )


def _emit_gather(nc, out_ap, in_ap, idxs_ap, num_idxs, elem_size,
                 elem_step, queue_num=0):
    """Emit InstDMAGatherAnt with elem_size_bytes that need not be a
    multiple of 256 (that bass-level assert is a transpose-mode
    restriction; the HW constraint is stride_bytes%256==0, which holds
    via elem_step). Mirrors BassGpSimd.dma_gather's lowering."""
    from concourse import mybir
    from concourse.ap_utils import ap_is_contiguous

    eng = nc.gpsimd
    assert idxs_ap.dtype == mybir.dt.int16
    assert in_ap.dtype == out_ap.dtype
    assert ap_is_contiguous(in_ap.ap[1:])
    assert ap_is_contiguous(out_ap.ap[1:])
    assert ap_is_contiguous(idxs_ap.ap[1:])
    assert in_ap.ap[-1][1] == out_ap.ap[-1][1] == elem_size
    assert out_ap.ap[0][1] * out_ap.ap[1][1] == -(-num_idxs // 128) * 128
    assert in_ap.ap[0][0] == elem_step
    stride_bytes = elem_step * mybir.dt.size(in_ap.dtype)
    assert stride_bytes % 256 == 0 and stride_bytes // 256 < 256
    _in_ap = eng.lower_ap_dma(in_ap, for_custom_bir_dma=True)
    _idxs_ap = eng.lower_ap(idxs_ap)
    _out_ap = eng.lower_ap(out_ap)
    return eng.add_instruction(
        mybir.InstDMAGatherAnt(
            name=nc.get_next_instruction_name(),
            ins=[
                *_in_ap,
                _idxs_ap,
                eng.lower_val_access(eng.to_reg(num_idxs)),
            ],
            outs=[_out_ap],
            transpose=False,
            num_idxs=num_idxs,
            elem_size=elem_size,
            stride_bytes_256=stride_bytes // 256,
            gen_mode=0,
            single_packet=False,
            queue_num=queue_num,
            sbuf_tokens_per_rank=0,
            sbuf_free_dim_per_rank=0,
            sbuf_free_dim_pad_per_rank=0,
            sbuf_byte_offset=0,
        )
    )


def _build_program(cfg, prep):
    from concourse import bacc, mybir, tile

    f, f2 = cfg.f, cfg.f2
    L, NEV, idx_cols = prep["L"], prep["NEV"], prep["idx_cols"]
    run_meta = prep["run_meta"]
    evprog = prep["evprog"]
    ev_ri, ev_g, ev_b = prep["ev_ri"], prep["ev_g"], prep["ev_b"]
    ev_start, ev_stop = prep["ev_start"], prep["ev_stop"]
    fp32 = mybir.dt.float32
    bf16 = mybir.dt.bfloat16
    Alu = mybir.AluOpType
    Act = mybir.ActivationFunctionType
    tdt = bf16 if BF16TAB else fp32
    trow = TPAD if BF16TAB else f

    # events grouped per (run, call)
    ev_by_call = {}
    for e in evprog:
        ri = int(ev_ri[e])
        g = int(ev_g[e])
        k = (g * P) // CALLMAX
        gc = g - k * (CALLMAX // P)
        ev_by_call.setdefault((ri, k), []).append(
            (gc, int(ev_b[e]), int(e), bool(ev_start[e]), bool(ev_stop[e]))
        )

    nc = bacc.Bacc("TRN2", target_bir_lowering=False, debug=False,
                   num_devices=N_CORES, dynamic_dma_scratch_size=SCRATCH,
                   num_swdge_queues=NQUEUES)

    xt_in = nc.dram_tensor("xt", [f, SHARD], fp32, kind="ExternalInput")
    w1_in = nc.dram_tensor("w1", [f, f], fp32, kind="ExternalInput")
    w2_in = nc.dram_tensor("w2", [f, f2], fp32, kind="ExternalInput")
    b1_in = nc.dram_tensor("b1r", [P, f], fp32, kind="ExternalInput")
    b2_in = nc.dram_tensor("b2r", [P, f2], fp32, kind="ExternalInput")
    idx_in = nc.dram_tensor("idx", [P, idx_cols], mybir.dt.int16,
                            kind="ExternalInput")
    colrel_in = nc.dram_tensor("colrel", [P, NEV], fp32, kind="ExternalInput")
    wsel_in = nc.dram_tensor("wsel", [P, NEV], fp32, kind="ExternalInput")
    ewdeg_in = nc.dram_tensor("ewdeg", [P, NB * L], fp32,
                              kind="ExternalInput")
    out_t = nc.dram_tensor("out", [SHARD, f2], fp32, kind="ExternalOutput")

    xw1_shard = nc.dram_tensor("xw1_shard", [SHARD, trow], tdt,
                               kind="Internal")
    xw1_full = nc.dram_tensor("xw1_full", [NPAD, trow], tdt, kind="Internal",
                              addr_space="Shared")
    h2_shard = nc.dram_tensor("h2_shard", [SHARD, trow], tdt, kind="Internal")
    h2_full = nc.dram_tensor("h2_full", [NPAD, trow], tdt, kind="Internal",
                             addr_space="Shared")

    rg = [list(range(N_CORES))]
    GW = CALLMAX // P          # max groups per call

    with tile.TileContext(nc) as tc:
        with tc.tile_pool(name="const", bufs=1) as cpool:
            # ---- constants ---------------------------------------------
            iota_i = cpool.tile([P, P], mybir.dt.int32, name="iota_i")
            nc.gpsimd.iota(iota_i[:], pattern=[[1, P]], base=0,
                           channel_multiplier=0)
            iota_b = cpool.tile([P, P], bf16, name="iota_b")
            nc.vector.tensor_copy(out=iota_b[:], in_=iota_i[:])
            pidx_i = cpool.tile([P, 1], mybir.dt.int32, name="pidx_i")
            nc.gpsimd.iota(pidx_i[:], pattern=[[0, 1]], base=0,
                           channel_multiplier=1)
            pidx_f = cpool.tile([P, 1], fp32, name="pidx_f")
            nc.vector.tensor_copy(out=pidx_f[:], in_=pidx_i[:])
            ident_b = cpool.tile([P, P], bf16, name="ident_b")
            nc.vector.tensor_scalar(out=ident_b[:], in0=iota_b[:],
                                    scalar1=pidx_f[:, 0:1], scalar2=None,
                                    op0=Alu.is_equal)
            w1_sb = cpool.tile([f, f], fp32, name="w1_sb")
            nc.sync.dma_start(out=w1_sb[:], in_=w1_in[:])
            w1_bf = cpool.tile([f, f], bf16, name="w1_bf")
            nc.vector.tensor_copy(out=w1_bf[:], in_=w1_sb[:])
            w2_sb = cpool.tile([f, f2], fp32, name="w2_sb")
            nc.sync.dma_start(out=w2_sb[:], in_=w2_in[:])
            w2_bf = cpool.tile([f, f2], bf16, name="w2_bf")
            nc.vector.tensor_copy(out=w2_bf[:], in_=w2_sb[:])
            b1_sb = cpool.tile([P, f], fp32, name="b1_sb")
            nc.sync.dma_start(out=b1_sb[:], in_=b1_in[:])
            b2_sb = cpool.tile([P, f2], fp32, name="b2_sb")
            nc.sync.dma_start(out=b2_sb[:], in_=b2_in[:])
            idx_sb = cpool.tile([P, idx_cols], mybir.dt.int16, name="idx_sb")
            nc.sync.dma_start(out=idx_sb[:], in_=idx_in[:])
            colrel_sb = cpool.tile([P, NEV], fp32, name="colrel_sb")
            nc.sync.dma_start(out=colrel_sb[:], in_=colrel_in[:])
            wsel_sb = cpool.tile([P, NEV], fp32, name="wsel_sb")
            nc.sync.dma_start(out=wsel_sb[:], in_=wsel_in[:])
            # negated copies for Act-engine masks: y=Abs(iota-c);
            # mask=Relu(negw*y + w)
            ncol_sb = cpool.tile([P, NEV], fp32, name="ncol_sb")
            nc.vector.tensor_scalar(out=ncol_sb[:], in0=colrel_sb[:],
                                    scalar1=-1.0, scalar2=None, op0=Alu.mult)
            nwsel_sb = cpool.tile([P, NEV], fp32, name="nwsel_sb")
            nc.vector.tensor_scalar(out=nwsel_sb[:], in0=wsel_sb[:],
                                    scalar1=-1.0, scalar2=None, op0=Alu.mult)
            dinv_sb = cpool.tile([P, NB], fp32, name="dinv_sb")
            out_stage = cpool.tile([P, NB * f2], fp32, name="out_stage")

            # ---- deg -> dinv (core-local) ------------------------------
            with tc.tile_pool(name="deg", bufs=1) as degp:
                ewdeg_sb = degp.tile([P, NB * L], fp32)
                nc.sync.dma_start(out=ewdeg_sb[:], in_=ewdeg_in[:])
                deg_sb = degp.tile([P, NB], fp32)
                for b in range(NB):
                    nc.vector.reduce_sum(
                        out=deg_sb[:, b:b + 1],
                        in_=ewdeg_sb[:, b * L:(b + 1) * L],
                        axis=mybir.AxisListType.X)
                sq_sb = degp.tile([P, NB], fp32)
                nc.scalar.activation(out=sq_sb[:], in_=deg_sb[:],
                                     func=Act.Sqrt)
                nc.vector.reciprocal(out=dinv_sb[:], in_=sq_sb[:])

            # ---- dense: table1 = dinv * (X @ W1) for my shard ----------
            SCH = 14
            with tc.tile_pool(name="xt", bufs=1) as xtp, \
                 tc.tile_pool(name="dps", bufs=4, space="PSUM") as dpp, \
                 tc.tile_pool(name="dst", bufs=2) as dstp:
                xtf = xtp.tile([f, SHARD], fp32, name="xtf")
                nc.sync.dma_start(out=xtf[:], in_=xt_in[:])
                xtb = xtp.tile([f, SHARD], bf16, name="xtb")
                nc.scalar.activation(out=xtb[:], in_=xtf[:], func=Act.Copy)
                for t0 in range(0, NB, SCH):
                    stg = dstp.tile([P, SCH * f], tdt, tag="dstg",
                                    name="dstg")
                    for j in range(SCH):
                        b = t0 + j
                        ps = dpp.tile([P, f], fp32, tag="dps", name="dps")
                        nc.tensor.matmul(out=ps[:],
                                         lhsT=xtb[:, b * P:(b + 1) * P],
                                         rhs=w1_bf[:], start=True, stop=True)
                        nc.vector.tensor_scalar(
                            out=stg[:, j * f:(j + 1) * f], in0=ps[:],
                            scalar1=dinv_sb[:, b:b + 1], scalar2=None,
                            op0=Alu.mult)
                    dst_ap = xw1_shard[t0 * P:(t0 + SCH) * P, :f].rearrange(
                        "(i p) f -> p i f", p=P)
                    nc.sync.dma_start(out=dst_ap, in_=stg[:])

            for j in range(4):
                nc.gpsimd.collective_compute(
                    "AllGather", Alu.bypass, replica_groups=rg,
                    ins=[xw1_shard[j * QROWS:(j + 1) * QROWS, :]],
                    outs=[xw1_full[j * CROWS:(j + 1) * CROWS, :]])

            # ---- aggregation layers ------------------------------------
            with tc.tile_pool(name="gst", bufs=4) as gpool, \
                 tc.tile_pool(name="gbf", bufs=3) as bpool, \
                 tc.tile_pool(name="mask", bufs=10) as mpool, \
                 tc.tile_pool(name="work", bufs=4) as wpool:

                def agg_layer(layer, shard_t, full_t):
                    # One PSUM bank per open dest block (HW allows only one
                    # live accumulation group per bank), <=7 open + head.
                    pss = {}                  # b -> psum tile
                    callno = [0]
                    evno = [0]

                    def finish_block(b):
                        ps = pss.pop(b)
                        # self-loop: identity-mask matmul on this block's
                        # own table rows (affine DMA, no gather slot)
                        srow = wpool.tile([P, f], tdt, tag="srow",
                                          name="srow")
                        nc.sync.dma_start(
                            out=srow[:],
                            in_=shard_t[b * P:(b + 1) * P, :f])
                        if layer == 1:
                            nc.tensor.matmul(out=ps[:], lhsT=ident_b[:],
                                             rhs=srow[:], start=False,
                                             stop=True)
                        else:
                            nc.tensor.matmul(out=ps[:], lhsT=srow[:],
                                             rhs=ident_b[:], start=False,
                                             stop=True)
                        if layer == 1:
                            pblk = ps[:]
                            t1 = wpool.tile([P, f], fp32, tag="t1",
                                            name="t1")
                            nc.vector.tensor_scalar(
                                out=t1[:], in0=pblk,
                                scalar1=dinv_sb[:, b:b + 1], scalar2=None,
                                op0=Alu.mult)
                            t2 = wpool.tile([P, f], fp32, tag="t2",
                                            name="t2")
                            nc.vector.tensor_tensor(
                                out=t2[:], in0=t1[:], in1=b1_sb[:],
                                op=Alu.add)
                            h2w = wpool.tile([P, f], tdt, tag="h2w",
                                             name="h2w")
                            # dinv>0 so relu(dinv*x) == dinv*relu(x)
                            nc.scalar.activation(
                                out=h2w[:], in_=t2[:], func=Act.Relu,
                                scale=dinv_sb[:, b:b + 1])
                            nc.sync.dma_start(
                                out=h2_shard[b * P:(b + 1) * P, :f],
                                in_=h2w[:])
                        else:
                            pblk = ps[:]
                            lh = wpool.tile([f, P], bf16, tag="lh",
                                            name="lh")
                            nc.vector.tensor_copy(out=lh[:], in_=pblk)
                            ps2 = pph.tile([P, f2], fp32, tag="head",
                                           name="ps2")
                            nc.tensor.matmul(out=ps2[:], lhsT=lh[:],
                                             rhs=w2_bf[:], start=True,
                                             stop=True)
                            t3 = wpool.tile([P, f2], fp32, tag="t3",
                                            name="t3")
                            nc.vector.tensor_scalar(
                                out=t3[:], in0=ps2[:],
                                scalar1=dinv_sb[:, b:b + 1], scalar2=None,
                                op0=Alu.mult)
                            nc.vector.tensor_tensor(
                                out=out_stage[:, b * f2:(b + 1) * f2],
                                in0=t3[:], in1=b2_sb[:], op=Alu.add)

                    def build_mask(e):
                        mask = mpool.tile([P, P], bf16, tag="mask",
                                          name="mask")
                        evno[0] += 1
                        if (evno[0] * ACTPCT) // 100 != \
                           ((evno[0] - 1) * ACTPCT) // 100:
                            # Activation-engine build (2 ops)
                            ytmp = mpool.tile([P, P], bf16, tag="ytmp",
                                              name="ytmp")
                            nc.scalar.activation(
                                out=ytmp[:], in_=iota_b[:], func=Act.Abs,
                                bias=ncol_sb[:, e:e + 1])
                            nc.scalar.activation(
                                out=mask[:], in_=ytmp[:], func=Act.Relu,
                                scale=nwsel_sb[:, e:e + 1],
                                bias=wsel_sb[:, e:e + 1])
                        else:
                            nc.vector.tensor_scalar(
                                out=mask[:], in0=iota_b[:],
                                scalar1=colrel_sb[:, e:e + 1],
                                scalar2=wsel_sb[:, e:e + 1],
                                op0=Alu.is_equal, op1=Alu.mult)
                        return mask

                    for ri, (q, s, base, nsl, calls) in enumerate(run_meta):
                        if s == 0:
                            table = shard_t[:, :f]
                        else:
                            table = full_t[(s - 1) * CROWS:s * CROWS, :f]
                        for k, (cbase, nn) in enumerate(calls):
                            gt = gpool.tile([P, GW * f], tdt, tag="gst",
                                            name=f"gt{layer}_{ri}_{k}")
                            if BF16TAB:
                                _emit_gather(
                                    nc,
                                    gt[:, :nn // P * f].rearrange(
                                        "p (a q) -> p a q", q=f),
                                    table,
                                    idx_sb[:, cbase // 16:(cbase + nn) // 16],
                                    nn, f, TPAD,
                                    queue_num=callno[0] % NQUEUES)
                            else:
                                nc.gpsimd.dma_gather(
                                    gt[:, :nn // P * f].rearrange(
                                        "p (a q) -> p a q", q=f),
                                    table,
                                    idx_sb[:, cbase // 16:(cbase + nn) // 16],
                                    nn, nn, f,
                                    single_packet=False,
                                    queue_num=callno[0] % NQUEUES)
                            callno[0] += 1
                            if BF16TAB:
                                gb_t = gt
                            else:
                                gb_t = bpool.tile([P, GW * f], bf16,
                                                  tag="gbf",
                                                  name=f"gb{layer}_{ri}_{k}")
                                nc.scalar.activation(
                                    out=gb_t[:, :nn // P * f],
                                    in_=gt[:, :nn // P * f], func=Act.Copy)
                            for gc, b, e, st, sp in ev_by_call.get(
                                    (ri, k), []):
                                mask = build_mask(e)
                                msg = gb_t[:, gc * f:(gc + 1) * f]
                                if b not in pss:
                                    if layer == 1:
                                        pss[b] = pp.tile([P, f], fp32,
                                                         tag="agg",
                                                         name="aps")
                                    else:
                                        pss[b] = pp.tile([f, P], fp32,
                                                         tag="agg",
                                                         name="apsT")
                                    st = True
                                if layer == 1:
                                    nc.tensor.matmul(out=pss[b][:],
                                                     lhsT=mask[:], rhs=msg,
                                                     start=st, stop=False)
                                else:
                                    nc.tensor.matmul(out=pss[b][:],
                                                     lhsT=msg, rhs=mask[:],
                                                     start=st, stop=False)
                                if sp:
                                    finish_block(b)
                    assert not pss, list(pss)

                with tc.tile_pool(name="agg1", bufs=SEPT,
                                  space="PSUM") as pp:
                    agg_layer(1, xw1_shard, xw1_full)

                for j in range(4):
                    nc.gpsimd.collective_compute(
                        "AllGather", Alu.bypass, replica_groups=rg,
                        ins=[h2_shard[j * QROWS:(j + 1) * QROWS, :]],
                        outs=[h2_full[j * CROWS:(j + 1) * CROWS, :]])

                with tc.tile_pool(name="agg2", bufs=SEPT,
                                  space="PSUM") as pp, \
                     tc.tile_pool(name="head", bufs=1, space="PSUM") as pph:
                    agg_layer(2, h2_shard, h2_full)

            out_ap = out_t[:].rearrange("(b p) f -> p b f", p=P)
            nc.sync.dma_start(out=out_ap, in_=out_stage[:])

    nc.compile()
    return nc


def _make_in_maps(cfg, prep, W1, b1, W2, b2):
    b1r = np.broadcast_to(np.asarray(b1, np.float32), (P, cfg.f)).copy()
    b2r = np.broadcast_to(np.asarray(b2, np.float32), (P, cfg.f2)).copy()
    w1 = np.asarray(W1, np.float32)
    w2 = np.asarray(W2, np.float32)
    in_maps = []
    for c in range(N_CORES):
        in_maps.append({
            "xt": prep["xt_shards"][c],
            "w1": w1, "w2": w2, "b1r": b1r, "b2r": b2r,
            "idx": np.ascontiguousarray(prep["idx16"][c]),
            "colrel": np.ascontiguousarray(prep["colrel_t"][c]),
            "wsel": np.ascontiguousarray(prep["wsel_t"][c]),
            "ewdeg": np.ascontiguousarray(prep["ewdeg"][c]),
        })
    return in_maps


def run(cfg, in_feat, edge_index, edge_weight, W1, b1, W2, b2,
        trace=False, use_sim=False):
    """Returns (output [n_real, f2], BassKernelResults|None)."""
    _install_ntff_shim()
    from concourse import bass_utils

    prep = _host_prep(cfg, in_feat, edge_index, edge_weight)
    nc = _build_program(cfg, prep)
    in_maps = _make_in_maps(cfg, prep, W1, b1, W2, b2)

    if use_sim:
        from concourse.bass_interp import MultiCoreSim
        sim = MultiCoreSim(nc, num_cores=N_CORES)
        for c, (cid, core) in enumerate(sim.cores.items()):
            for k, v in in_maps[c].items():
                core.tensor(k)[:] = v
        sim.simulate()
        shards = [sim.cores[c].tensor("out").copy() for c in sim.cores]
        res = None
    else:
        res = bass_utils.run_bass_kernel_spmd(
            nc, in_maps, core_ids=list(range(N_CORES)), trace=trace)
        shards = [res.results[c]["out"] for c in range(N_CORES)]

    out_perm = np.concatenate(shards, axis=0)  # [npad, f2]
    out = out_perm[prep["nid"][:cfg.n_real]]
    return out, res


def kernel(in_feat, edge_index, edge_weight, W1, b1, W2, b2):
    cfg = Cfg(n_real=100000, f_in=64, f_out=16, blocks_per_core=98)
    out, _ = run(cfg, in_feat, edge_index, edge_weight, W1, b1, W2, b2)
    return np.ascontiguousarray(out.astype(np.float32))


# revision 25
# speedup vs baseline: 1.2818x; 1.0151x over previous
"""Trainium2 Bass kernel for a 2-layer GCN (gnn_message_passing) — v3.

Strategy (8 NeuronCores, SPMD). Profile-driven rebuild of v2:
  - The v2 bottleneck was GPSIMD: dma_gather costs ~5-8ns/index + ~3us
    fixed of Pool-engine time (Q7 descriptor-gen), so v3 uses far fewer,
    far larger gather calls: K_CALLMAX=4096 (one call per run instead of
    ~8) -> 70 calls/layer instead of 266. Calls >1024 indices require
    single_packet=False: with single_packet=True the whole per-engine
    descriptor stream coalesces into ONE DMA packet, and 1024 idx x 256B
    / 16 engines = 16KB is the packet-size ceiling (bigger crashes).
  - PSUM: one bank per open dest block (HW allows only ONE live matmul
    accumulation group per bank — interleaving two groups in one bank
    drops accumulations; verified by micro-test), so SEPT=7 + head.
  - Tables stored bf16 padded to 256B rows ([N,128] bf16, features in
    cols 0..63). dma_gather descriptors then move 128B/row instead of
    256B (elem_size=64, elem_step=128 -> stride 256B satisfies the HW
    stride%256 constraint; the bass-level elem_size_bytes%256 assert is a
    transpose-mode restriction, bypassed by emitting InstDMAGatherAnt
    directly). Halves gather HBM traffic and kills the fp32->bf16 cast.
  - Scatter masks (weighted one-hot [slot x dest] bf16) are built on DVE
    (tensor_scalar is_eq*mult) and optionally a fraction on the idle
    Activation engine via two activations: y=Abs(iota-c) then
    mask=Relu(w - w*y)  (K_ACTMASK_PCT).
  - Dest nodes balance-permuted, sharded 8 x (98 blocks x 128); weights
    replicated; per-layer tables (dinv-prescaled transformed features)
    produced shard-wise and AllGathered; 'own' runs gather from the local
    shard table to hide AllGather latency.
"""

import sys
import types

if "/opt/trn_rl_repo" not in sys.path:
    sys.path.insert(0, "/opt/trn_rl_repo")

import numpy as np


def _install_ntff_shim():
    """antenv.axon_hooks is missing in this container; provide it so
    trace=True (NTFF profiling) works under axon."""
    if "antenv.axon_hooks" in sys.modules:
        return
    try:
        import antenv  # noqa: F401
    except ImportError:
        return
    shim = types.ModuleType("antenv.axon_hooks")
    shim._hook = None

    def set_axon_ntff_profile_hook(h):
        shim._hook = h

    def get_axon_ntff_profile_hook():
        return shim._hook

    shim.set_axon_ntff_profile_hook = set_axon_ntff_profile_hook
    shim.get_axon_ntff_profile_hook = get_axon_ntff_profile_hook
    sys.modules["antenv.axon_hooks"] = shim
    import antenv

    antenv.axon_hooks = shim
    try:
        from trn_agent_boot.trn_boot import _ntff_profile_via_ctypes

        shim._hook = _ntff_profile_via_ctypes("/opt/axon/libaxon_pjrt.so")
    except Exception:
        pass


import os

N_CORES = 8
P = 128
NB = 98                     # dest blocks per core
SHARD = NB * P              # 12544
NPAD = N_CORES * SHARD      # 100352
CROWS = NPAD // 4           # 25088 rows per global chunk (int16-safe)
QROWS = SHARD // 4          # 3136: shard quarter; chunk j is the
                            # rank-concat of all cores' quarter j, so one
                            # quarter-slice AllGather produces chunk j and
                            # global runs only wait for their own chunk
SEPT = int(os.environ.get("K_SEPT", "7"))    # blocks per dest group
# HW: only ONE live matmul accumulation group per PSUM bank (verified by
# micro-test: interleaved start/accum in one bank drops accumulations), so
# every open dest block needs its own bank: SEPT+head <= 8.
assert SEPT <= 7
NSEPT = NB // SEPT
assert NSEPT * SEPT == NB
NSRC = 5                    # source tables: 0=own shard, 1..4=global chunks
CALLMAX = int(os.environ.get("K_CALLMAX", "4096"))   # gather idxs per call
SCRATCH = int(os.environ.get("K_SCRATCH", "16384"))  # SWDGE ring bytes
NQUEUES = int(os.environ.get("K_NQUEUES", "1"))      # SWDGE queues
BF16TAB = int(os.environ.get("K_BF16TAB", "1"))      # bf16 padded tables
ACTPCT = int(os.environ.get("K_ACTMASK_PCT", "50"))  # % masks on Act engine
TPAD = 128                  # padded table row (bf16 elems) = 256B stride
OWN_FROM_FULL = False       # debug: route own edges via the global table


class Cfg:
    def __init__(self, n_real, f_in, f_out, blocks_per_core, sb_blocks=7):
        assert blocks_per_core == NB
        self.n_real = n_real
        self.f = f_in
        self.f2 = f_out
        self.nb = blocks_per_core
        self.shard = SHARD
        self.npad = NPAD
        assert self.npad >= n_real


def _host_prep(cfg, in_feat, edge_index, edge_weight):
    """Index/layout preprocessing (numpy only; all float math on device
    except copying edge weights into mask-scalar layouts)."""
    n, f = in_feat.shape
    assert n == cfg.n_real and f == cfg.f

    src = np.asarray(edge_index[0], dtype=np.int64)
    dst = np.asarray(edge_index[1], dtype=np.int64)
    ew = np.asarray(edge_weight, dtype=np.float32)
    loop = np.arange(n, dtype=np.int64)      # self loops, weight 1
    # Self-loops are NOT gathered per-edge: the self contribution is an
    # identity-mask matmul on the dest block's own table rows (affine DMA)
    # issued in finish_block. They still count for deg.
    dst_deg = np.concatenate([dst, loop])
    ew_deg = np.concatenate([ew, np.ones(n, np.float32)])

    # ---- balance-permute dest nodes into blocks of 128 slots -----------
    cnt = np.bincount(dst_deg, minlength=NPAD)
    order = np.argsort(-cnt, kind="stable")
    pos = np.arange(NPAD, dtype=np.int64)
    nblocks = N_CORES * NB
    pas = pos // nblocks
    bin_idx = pos % nblocks
    odd = (pas % 2) == 1
    bin_idx[odd] = nblocks - 1 - bin_idx[odd]
    nid = np.empty(NPAD, dtype=np.int64)
    nid[order] = bin_idx * P + pas

    dst_n = nid[dst]
    src_n = nid[src]
    dst_ndeg = nid[dst_deg]
    L = int(cnt.max())

    core_e = dst_n // SHARD
    b_e = (dst_n % SHARD) // P
    q_e = b_e // SEPT                        # dest group
    src_core = src_n // SHARD
    own = src_core == core_e
    if OWN_FROM_FULL:
        own[:] = False
    src_r = src_n % SHARD
    s_e = np.where(own, 0, src_r // QROWS + 1).astype(np.int64)
    tidx_e = np.where(own, src_r,
                      (src_n // SHARD) * QROWS + src_r % QROWS)

    # ---- max-over-cores segment sizes ----------------------------------
    cnt_bs = np.zeros((N_CORES, NB, NSRC), np.int64)
    np.add.at(cnt_bs, (core_e, b_e, s_e), 1)
    R = cnt_bs.max(axis=0)                   # [NB, NSRC]

    # ---- run / segment layout ------------------------------------------
    seg_off = np.zeros((NB, NSRC), np.int64)   # slot offset within run
    run_base_qs = np.zeros((NSEPT, NSRC), np.int64)
    run_meta = []                              # per ri: (q, s, base, nslots, calls)
    slot_base = 0
    for q in range(NSEPT):
        for s in range(NSRC):
            off = 0
            for b in range(q * SEPT, (q + 1) * SEPT):
                seg_off[b, s] = off
                off += R[b, s]
            nsl = -(-off // P) * P
            calls = []
            cb = 0
            while cb < nsl:
                nn = min(CALLMAX, nsl - cb)
                calls.append((slot_base + cb, nn))
                cb += nn
            run_base_qs[q, s] = slot_base
            run_meta.append((q, s, slot_base, nsl, calls))
            slot_base += nsl
    TOT = slot_base
    assert TOT % 16 == 0

    # ---- events: one mask+matmul per (run, 128-slot group, block) ------
    # Enumerated segment-major so per-edge event ids are arithmetic.
    first_eid = np.full((NB, NSRC), -1, np.int64)
    seg_g0 = np.zeros((NB, NSRC), np.int64)
    ev_ri = []
    ev_g = []
    ev_b = []
    for q in range(NSEPT):
        for s in range(NSRC):
            ri = q * NSRC + s
            for b in range(q * SEPT, (q + 1) * SEPT):
                if R[b, s] == 0:
                    continue
                g0 = seg_off[b, s] // P
                g1 = (seg_off[b, s] + R[b, s] - 1) // P
                first_eid[b, s] = len(ev_ri)
                seg_g0[b, s] = g0
                for g in range(g0, g1 + 1):
                    ev_ri.append(ri)
                    ev_g.append(g)
                    ev_b.append(b)
    NEV = len(ev_ri)
    ev_ri = np.array(ev_ri)
    ev_g = np.array(ev_g)
    ev_b = np.array(ev_b)

    ev_start = np.zeros(NEV, bool)
    ev_stop = np.zeros(NEV, bool)
    for b in range(NB):
        ss = [s for s in range(NSRC) if R[b, s] > 0]
        assert ss, f"block {b} has no edges?"
        ev_start[first_eid[b, ss[0]]] = True
        ls = ss[-1]
        g0 = seg_off[b, ls] // P
        g1 = (seg_off[b, ls] + R[b, ls] - 1) // P
        ev_stop[first_eid[b, ls] + (g1 - g0)] = True

    # program iteration order: by (run, group, block)
    evprog = np.lexsort((ev_b, ev_g, ev_ri))

    # ---- per-edge slot / event / partition -----------------------------
    segkey = (core_e * NB + b_e) * NSRC + s_e
    eorder = np.argsort(segkey, kind="stable")
    key_s = segkey[eorder]
    seg_start = np.searchsorted(key_s, np.arange(N_CORES * NB * NSRC))
    rank = np.arange(len(key_s)) - seg_start[key_s]
    core_s = core_e[eorder]
    b_s = b_e[eorder]
    s_s = s_e[eorder]
    q_s = q_e[eorder]
    dst_s = dst_n[eorder]
    ew_s = ew[eorder]
    tidx_s = tidx_e[eorder]

    slotrel = seg_off[b_s, s_s] + rank          # slot within run
    slot = run_base_qs[q_s, s_s] + slotrel      # global slot
    gi_run = slotrel // P
    p_slot = slotrel % P
    eid = first_eid[b_s, s_s] + (gi_run - seg_g0[b_s, s_s])

    colrel_t = np.zeros((N_CORES, P, NEV), np.float32)
    wsel_t = np.zeros((N_CORES, P, NEV), np.float32)
    colrel_t[core_s, p_slot, eid] = (dst_s % P).astype(np.float32)
    wsel_t[core_s, p_slot, eid] = ew_s

    # ---- int16 gather indices (16-partition wrap, replicated to 128) ---
    idx_cols = TOT // 16
    idx16 = np.zeros((N_CORES, 16, idx_cols), np.int16)
    idx16[core_s, slot % 16, slot // 16] = tidx_s.astype(np.int16)
    idx16 = np.broadcast_to(
        idx16[:, None, :, :], (N_CORES, 8, 16, idx_cols)
    ).reshape(N_CORES, P, idx_cols)

    # ---- deg layout (device computes deg = row-sum, dinv = 1/sqrt) -----
    dorder = np.argsort(dst_ndeg, kind="stable")
    dst_d = dst_ndeg[dorder]
    ew_d = ew_deg[dorder]
    dfirst = np.searchsorted(dst_d, np.arange(NPAD))
    rankd = np.arange(len(dst_d)) - dfirst[dst_d]
    assert rankd.max() < L
    ewdeg = np.zeros((N_CORES, P, NB * L), np.float32)
    ewdeg[dst_d // SHARD, dst_d % P, ((dst_d % SHARD) // P) * L + rankd] = ew_d
    zdeg = np.where(cnt == 0)[0]
    if len(zdeg) > 0:
        zn = nid[zdeg]
        ewdeg[zn // SHARD, zn % P, ((zn % SHARD) // P) * L] = 1.0

    # ---- permuted, padded, transposed features -------------------------
    xperm = np.zeros((NPAD, f), np.float32)
    xperm[nid[:n]] = np.asarray(in_feat, np.float32)
    xt_shards = [
        np.ascontiguousarray(xperm[c * SHARD:(c + 1) * SHARD].T)
        for c in range(N_CORES)
    ]

    return dict(
        L=L, nid=nid, xt_shards=xt_shards, NEV=NEV, TOT=TOT,
        colrel_t=colrel_t, wsel_t=wsel_t, ewdeg=ewdeg, idx16=idx16,
        idx_cols=idx_cols, run_meta=run_meta, evprog=evprog,
        ev_ri=ev_ri, ev_g=ev_g, ev_b=ev_b, ev_start=ev_start,
        ev_stop=ev_stop,
    )


def _emit_gather(nc, out_ap, in_ap, idxs_ap, num_idxs, elem_size,
                 elem_step, queue_num=0):
    """Emit InstDMAGatherAnt with elem_size_bytes that need not be a
    multiple of 256 (that bass-level assert is a transpose-mode
    restriction; the HW constraint is stride_bytes%256==0, which holds
    via elem_step). Mirrors BassGpSimd.dma_gather's lowering."""
    from concourse import mybir
    from concourse.ap_utils import ap_is_contiguous

    eng = nc.gpsimd
    assert idxs_ap.dtype == mybir.dt.int16
    assert in_ap.dtype == out_ap.dtype
    assert ap_is_contiguous(in_ap.ap[1:])
    assert ap_is_contiguous(out_ap.ap[1:])
    assert ap_is_contiguous(idxs_ap.ap[1:])
    assert in_ap.ap[-1][1] == out_ap.ap[-1][1] == elem_size
    assert out_ap.ap[0][1] * out_ap.ap[1][1] == -(-num_idxs // 128) * 128
    assert in_ap.ap[0][0] == elem_step
    stride_bytes = elem_step * mybir.dt.size(in_ap.dtype)
    assert stride_bytes % 256 == 0 and stride_bytes // 256 < 256
    _in_ap = eng.lower_ap_dma(in_ap, for_custom_bir_dma=True)
    _idxs_ap = eng.lower_ap(idxs_ap)
    _out_ap = eng.lower_ap(out_ap)
    return eng.add_instruction(
        mybir.InstDMAGatherAnt(
            name=nc.get_next_instruction_name(),
            ins=[
                *_in_ap,
                _idxs_ap,
                eng.lower_val_access(eng.to_reg(num_idxs)),
            ],
            outs=[_out_ap],
            transpose=False,
            num_idxs=num_idxs,
            elem_size=elem_size,
            stride_bytes_256=stride_bytes // 256,
            gen_mode=0,
            single_packet=False,
            queue_num=queue_num,
            sbuf_tokens_per_rank=0,
            sbuf_free_dim_per_rank=0,
            sbuf_free_dim_pad_per_rank=0,
            sbuf_byte_offset=0,
        )
    )


def _build_program(cfg, prep):
    from concourse import bacc, mybir, tile

    f, f2 = cfg.f, cfg.f2
    L, NEV, idx_cols = prep["L"], prep["NEV"], prep["idx_cols"]
    run_meta = prep["run_meta"]
    evprog = prep["evprog"]
    ev_ri, ev_g, ev_b = prep["ev_ri"], prep["ev_g"], prep["ev_b"]
    ev_start, ev_stop = prep["ev_start"], prep["ev_stop"]
    fp32 = mybir.dt.float32
    bf16 = mybir.dt.bfloat16
    Alu = mybir.AluOpType
    Act = mybir.ActivationFunctionType
    tdt = bf16 if BF16TAB else fp32
    trow = TPAD if BF16TAB else f

    # events grouped per (run, call)
    ev_by_call = {}
    for e in evprog:
        ri = int(ev_ri[e])
        g = int(ev_g[e])
        k = (g * P) // CALLMAX
        gc = g - k * (CALLMAX // P)
        ev_by_call.setdefault((ri, k), []).append(
            (gc, int(ev_b[e]), int(e), bool(ev_start[e]), bool(ev_stop[e]))
        )

    nc = bacc.Bacc("TRN2", target_bir_lowering=False, debug=False,
                   num_devices=N_CORES, dynamic_dma_scratch_size=SCRATCH,
                   num_swdge_queues=NQUEUES)

    xt_in = nc.dram_tensor("xt", [f, SHARD], fp32, kind="ExternalInput")
    w1_in = nc.dram_tensor("w1", [f, f], fp32, kind="ExternalInput")
    w2_in = nc.dram_tensor("w2", [f, f2], fp32, kind="ExternalInput")
    b1_in = nc.dram_tensor("b1r", [P, f], fp32, kind="ExternalInput")
    b2_in = nc.dram_tensor("b2r", [P, f2], fp32, kind="ExternalInput")
    idx_in = nc.dram_tensor("idx", [P, idx_cols], mybir.dt.int16,
                            kind="ExternalInput")
    colrel_in = nc.dram_tensor("colrel", [P, NEV], fp32, kind="ExternalInput")
    wsel_in = nc.dram_tensor("wsel", [P, NEV], fp32, kind="ExternalInput")
    ewdeg_in = nc.dram_tensor("ewdeg", [P, NB * L], fp32,
                              kind="ExternalInput")
    out_t = nc.dram_tensor("out", [SHARD, f2], fp32, kind="ExternalOutput")

    xw1_shard = nc.dram_tensor("xw1_shard", [SHARD, trow], tdt,
                               kind="Internal")
    xw1_full = nc.dram_tensor("xw1_full", [NPAD, trow], tdt, kind="Internal",
                              addr_space="Shared")
    h2_shard = nc.dram_tensor("h2_shard", [SHARD, trow], tdt, kind="Internal")
    h2_full = nc.dram_tensor("h2_full", [NPAD, trow], tdt, kind="Internal",
                             addr_space="Shared")

    rg = [list(range(N_CORES))]
    GW = CALLMAX // P          # max groups per call

    with tile.TileContext(nc) as tc:
        with tc.tile_pool(name="const", bufs=1) as cpool:
            # ---- constants ---------------------------------------------
            iota_i = cpool.tile([P, P], mybir.dt.int32, name="iota_i")
            nc.gpsimd.iota(iota_i[:], pattern=[[1, P]], base=0,
                           channel_multiplier=0)
            iota_b = cpool.tile([P, P], bf16, name="iota_b")
            nc.vector.tensor_copy(out=iota_b[:], in_=iota_i[:])
            pidx_i = cpool.tile([P, 1], mybir.dt.int32, name="pidx_i")
            nc.gpsimd.iota(pidx_i[:], pattern=[[0, 1]], base=0,
                           channel_multiplier=1)
            pidx_f = cpool.tile([P, 1], fp32, name="pidx_f")
            nc.vector.tensor_copy(out=pidx_f[:], in_=pidx_i[:])
            ident_b = cpool.tile([P, P], bf16, name="ident_b")
            nc.vector.tensor_scalar(out=ident_b[:], in0=iota_b[:],
                                    scalar1=pidx_f[:, 0:1], scalar2=None,
                                    op0=Alu.is_equal)
            w1_sb = cpool.tile([f, f], fp32, name="w1_sb")
            nc.sync.dma_start(out=w1_sb[:], in_=w1_in[:])
            w1_bf = cpool.tile([f, f], bf16, name="w1_bf")
            nc.vector.tensor_copy(out=w1_bf[:], in_=w1_sb[:])
            w2_sb = cpool.tile([f, f2], fp32, name="w2_sb")
            nc.sync.dma_start(out=w2_sb[:], in_=w2_in[:])
            w2_bf = cpool.tile([f, f2], bf16, name="w2_bf")
            nc.vector.tensor_copy(out=w2_bf[:], in_=w2_sb[:])
            b1_sb = cpool.tile([P, f], fp32, name="b1_sb")
            nc.sync.dma_start(out=b1_sb[:], in_=b1_in[:])
            b2_sb = cpool.tile([P, f2], fp32, name="b2_sb")
            nc.sync.dma_start(out=b2_sb[:], in_=b2_in[:])
            idx_sb = cpool.tile([P, idx_cols], mybir.dt.int16, name="idx_sb")
            nc.sync.dma_start(out=idx_sb[:], in_=idx_in[:])
            colrel_sb = cpool.tile([P, NEV], fp32, name="colrel_sb")
            nc.sync.dma_start(out=colrel_sb[:], in_=colrel_in[:])
            wsel_sb = cpool.tile([P, NEV], fp32, name="wsel_sb")
            nc.sync.dma_start(out=wsel_sb[:], in_=wsel_in[:])
            # negated copies for Act-engine masks: y=Abs(iota-c);
            # mask=Relu(negw*y + w)
            ncol_sb = cpool.tile([P, NEV], fp32, name="ncol_sb")
            nc.vector.tensor_scalar(out=ncol_sb[:], in0=colrel_sb[:],
                                    scalar1=-1.0, scalar2=None, op0=Alu.mult)
            nwsel_sb = cpool.tile([P, NEV], fp32, name="nwsel_sb")
            nc.vector.tensor_scalar(out=nwsel_sb[:], in0=wsel_sb[:],
                                    scalar1=-1.0, scalar2=None, op0=Alu.mult)
            dinv_sb = cpool.tile([P, NB], fp32, name="dinv_sb")
            out_stage = cpool.tile([P, NB * f2], fp32, name="out_stage")

            # ---- deg -> dinv (core-local) ------------------------------
            with tc.tile_pool(name="deg", bufs=1) as degp:
                ewdeg_sb = degp.tile([P, NB * L], fp32)
                nc.sync.dma_start(out=ewdeg_sb[:], in_=ewdeg_in[:])
                deg_sb = degp.tile([P, NB], fp32)
                for b in range(NB):
                    nc.vector.reduce_sum(
                        out=deg_sb[:, b:b + 1],
                        in_=ewdeg_sb[:, b * L:(b + 1) * L],
                        axis=mybir.AxisListType.X)
                sq_sb = degp.tile([P, NB], fp32)
                nc.scalar.activation(out=sq_sb[:], in_=deg_sb[:],
                                     func=Act.Sqrt)
                nc.vector.reciprocal(out=dinv_sb[:], in_=sq_sb[:])

            # ---- dense: table1 = dinv * (X @ W1) for my shard ----------
            SCH = 14
            with tc.tile_pool(name="xt", bufs=1) as xtp, \
                 tc.tile_pool(name="dps", bufs=4, space="PSUM") as dpp, \
                 tc.tile_pool(name="dst", bufs=2) as dstp:
                xtf = xtp.tile([f, SHARD], fp32, name="xtf")
                nc.sync.dma_start(out=xtf[:], in_=xt_in[:])
                xtb = xtp.tile([f, SHARD], bf16, name="xtb")
                nc.scalar.activation(out=xtb[:], in_=xtf[:], func=Act.Copy)
                for t0 in range(0, NB, SCH):
                    stg = dstp.tile([P, SCH * f], tdt, tag="dstg",
                                    name="dstg")
                    for j in range(SCH):
                        b = t0 + j
                        ps = dpp.tile([P, f], fp32, tag="dps", name="dps")
                        nc.tensor.matmul(out=ps[:],
                                         lhsT=xtb[:, b * P:(b + 1) * P],
                                         rhs=w1_bf[:], start=True, stop=True)
                        nc.vector.tensor_scalar(
                            out=stg[:, j * f:(j + 1) * f], in0=ps[:],
                            scalar1=dinv_sb[:, b:b + 1], scalar2=None,
                            op0=Alu.mult)
                    dst_ap = xw1_shard[t0 * P:(t0 + SCH) * P, :f].rearrange(
                        "(i p) f -> p i f", p=P)
                    nc.sync.dma_start(out=dst_ap, in_=stg[:])

            for j in range(4):
                nc.gpsimd.collective_compute(
                    "AllGather", Alu.bypass, replica_groups=rg,
                    ins=[xw1_shard[j * QROWS:(j + 1) * QROWS, :]],
                    outs=[xw1_full[j * CROWS:(j + 1) * CROWS, :]])

            # ---- aggregation layers ------------------------------------
            with tc.tile_pool(name="gst", bufs=4) as gpool, \
                 tc.tile_pool(name="gbf", bufs=3) as bpool, \
                 tc.tile_pool(name="mask", bufs=10) as mpool, \
                 tc.tile_pool(name="work", bufs=4) as wpool:

                def agg_layer(layer, shard_t, full_t):
                    # One PSUM bank per open dest block (HW allows only one
                    # live accumulation group per bank), <=7 open + head.
                    pss = {}                  # b -> psum tile
                    callno = [0]
                    evno = [0]

                    def finish_block(b):
                        ps = pss.pop(b)
                        # self-loop: identity-mask matmul on this block's
                        # own table rows (affine DMA, no gather slot)
                        srow = wpool.tile([P, f], tdt, tag="srow",
                                          name="srow")
                        nc.sync.dma_start(
                            out=srow[:],
                            in_=shard_t[b * P:(b + 1) * P, :f])
                        if layer == 1:
                            nc.tensor.matmul(out=ps[:], lhsT=ident_b[:],
                                             rhs=srow[:], start=False,
                                             stop=True)
                        else:
                            nc.tensor.matmul(out=ps[:], lhsT=srow[:],
                                             rhs=ident_b[:], start=False,
                                             stop=True)
                        if layer == 1:
                            pblk = ps[:]
                            t1 = wpool.tile([P, f], fp32, tag="t1",
                                            name="t1")
                            nc.vector.tensor_scalar(
                                out=t1[:], in0=pblk,
                                scalar1=dinv_sb[:, b:b + 1], scalar2=None,
                                op0=Alu.mult)
                            t2 = wpool.tile([P, f], fp32, tag="t2",
                                            name="t2")
                            nc.vector.tensor_tensor(
                                out=t2[:], in0=t1[:], in1=b1_sb[:],
                                op=Alu.add)
                            h2w = wpool.tile([P, f], tdt, tag="h2w",
                                             name="h2w")
                            # dinv>0 so relu(dinv*x) == dinv*relu(x)
                            nc.scalar.activation(
                                out=h2w[:], in_=t2[:], func=Act.Relu,
                                scale=dinv_sb[:, b:b + 1])
                            nc.sync.dma_start(
                                out=h2_shard[b * P:(b + 1) * P, :f],
                                in_=h2w[:])
                        else:
                            pblk = ps[:]
                            lh = wpool.tile([f, P], bf16, tag="lh",
                                            name="lh")
                            nc.vector.tensor_copy(out=lh[:], in_=pblk)
                            ps2 = pph.tile([P, f2], fp32, tag="head",
                                           name="ps2")
                            nc.tensor.matmul(out=ps2[:], lhsT=lh[:],
                                             rhs=w2_bf[:], start=True,
                                             stop=True)
                            t3 = wpool.tile([P, f2], fp32, tag="t3",
                                            name="t3")
                            nc.vector.tensor_scalar(
                                out=t3[:], in0=ps2[:],
                                scalar1=dinv_sb[:, b:b + 1], scalar2=None,
                                op0=Alu.mult)
                            nc.vector.tensor_tensor(
                                out=out_stage[:, b * f2:(b + 1) * f2],
                                in0=t3[:], in1=b2_sb[:], op=Alu.add)

                    def build_mask(e):
                        mask = mpool.tile([P, P], bf16, tag="mask",
                                          name="mask")
                        evno[0] += 1
                        if (evno[0] * ACTPCT) // 100 != \
                           ((evno[0] - 1) * ACTPCT) // 100:
                            # Activation-engine build (2 ops)
                            ytmp = mpool.tile([P, P], bf16, tag="ytmp",
                                              name="ytmp")
                            nc.scalar.activation(
                                out=ytmp[:], in_=iota_b[:], func=Act.Abs,
                                bias=ncol_sb[:, e:e + 1])
                            nc.scalar.activation(
                                out=mask[:], in_=ytmp[:], func=Act.Relu,
                                scale=nwsel_sb[:, e:e + 1],
                                bias=wsel_sb[:, e:e + 1])
                        else:
                            nc.vector.tensor_scalar(
                                out=mask[:], in0=iota_b[:],
                                scalar1=colrel_sb[:, e:e + 1],
                                scalar2=wsel_sb[:, e:e + 1],
                                op0=Alu.is_equal, op1=Alu.mult)
                        return mask

                    for ri, (q, s, base, nsl, calls) in enumerate(run_meta):
                        if s == 0:
                            table = shard_t[:, :f]
                        else:
                            table = full_t[(s - 1) * CROWS:s * CROWS, :f]
                        for k, (cbase, nn) in enumerate(calls):
                            gt = gpool.tile([P, GW * f], tdt, tag="gst",
                                            name=f"gt{layer}_{ri}_{k}")
                            if BF16TAB:
                                _emit_gather(
                                    nc,
                                    gt[:, :nn // P * f].rearrange(
                                        "p (a q) -> p a q", q=f),
                                    table,
                                    idx_sb[:, cbase // 16:(cbase + nn) // 16],
                                    nn, f, TPAD,
                                    queue_num=callno[0] % NQUEUES)
                            else:
                                nc.gpsimd.dma_gather(
                                    gt[:, :nn // P * f].rearrange(
                                        "p (a q) -> p a q", q=f),
                                    table,
                                    idx_sb[:, cbase // 16:(cbase + nn) // 16],
                                    nn, nn, f,
                                    single_packet=False,
                                    queue_num=callno[0] % NQUEUES)
                            callno[0] += 1
                            if BF16TAB:
                                gb_t = gt
                            else:
                                gb_t = bpool.tile([P, GW * f], bf16,
                                                  tag="gbf",
                                                  name=f"gb{layer}_{ri}_{k}")
                                nc.scalar.activation(
                                    out=gb_t[:, :nn // P * f],
                                    in_=gt[:, :nn // P * f], func=Act.Copy)
                            for gc, b, e, st, sp in ev_by_call.get(
                                    (ri, k), []):
                                mask = build_mask(e)
                                msg = gb_t[:, gc * f:(gc + 1) * f]
                                if b not in pss:
                                    if layer == 1:
                                        pss[b] = pp.tile([P, f], fp32,
                                                         tag="agg",
                                                         name="aps")
                                    else:
                                        pss[b] = pp.tile([f, P], fp32,
                                                         tag="agg",
                                                         name="apsT")
                                    st = True
                                if layer == 1:
                                    nc.tensor.matmul(out=pss[b][:],
                                                     lhsT=mask[:], rhs=msg,
                                                     start=st, stop=False)
                                else:
                                    nc.tensor.matmul(out=pss[b][:],
                                                     lhsT=msg, rhs=mask[:],
                                                     start=st, stop=False)
                                if sp:
                                    finish_block(b)
                    assert not pss, list(pss)

                with tc.tile_pool(name="agg1", bufs=SEPT,
                                  space="PSUM") as pp:
                    agg_layer(1, xw1_shard, xw1_full)

                for j in range(4):
                    nc.gpsimd.collective_compute(
                        "AllGather", Alu.bypass, replica_groups=rg,
                        ins=[h2_shard[j * QROWS:(j + 1) * QROWS, :]],
                        outs=[h2_full[j * CROWS:(j + 1) * CROWS, :]])

                with tc.tile_pool(name="agg2", bufs=SEPT,
                                  space="PSUM") as pp, \
                     tc.tile_pool(name="head", bufs=1, space="PSUM") as pph:
                    agg_layer(2, h2_shard, h2_full)

            out_ap = out_t[:].rearrange("(b p) f -> p b f", p=P)
            nc.sync.dma_start(out=out_ap, in_=out_stage[:])

    nc.compile()
    return nc


def _make_in_maps(cfg, prep, W1, b1, W2, b2):
    b1r = np.broadcast_to(np.asarray(b1, np.float32), (P, cfg.f)).copy()
    b2r = np.broadcast_to(np.asarray(b2, np.float32), (P, cfg.f2)).copy()
    w1 = np.asarray(W1, np.float32)
    w2 = np.asarray(W2, np.float32)
    in_maps = []
    for c in range(N_CORES):
        in_maps.append({
            "xt": prep["xt_shards"][c],
            "w1": w1, "w2": w2, "b1r": b1r, "b2r": b2r,
            "idx": np.ascontiguousarray(prep["idx16"][c]),
            "colrel": np.ascontiguousarray(prep["colrel_t"][c]),
            "wsel": np.ascontiguousarray(prep["wsel_t"][c]),
            "ewdeg": np.ascontiguousarray(prep["ewdeg"][c]),
        })
    return in_maps


def run(cfg, in_feat, edge_index, edge_weight, W1, b1, W2, b2,
        trace=False, use_sim=False):
    """Returns (output [n_real, f2], BassKernelResults|None)."""
    _install_ntff_shim()
    from concourse import bass_utils

    prep = _host_prep(cfg, in_feat, edge_index, edge_weight)
    nc = _build_program(cfg, prep)
    in_maps = _make_in_maps(cfg, prep, W1, b1, W2, b2)

    if use_sim:
        from concourse.bass_interp import MultiCoreSim
        sim = MultiCoreSim(nc, num_cores=N_CORES)
        for c, (cid, core) in enumerate(sim.cores.items()):
            for k, v in in_maps[c].items():
                core.tensor(k)[:] = v
        sim.simulate()
        shards = [sim.cores[c].tensor("out").copy() for c in sim.cores]
        res = None
    else:
        res = bass_utils.run_bass_kernel_spmd(
            nc, in_maps, core_ids=list(range(N_CORES)), trace=trace)
        shards = [res.results[c]["out"] for c in range(N_CORES)]

    out_perm = np.concatenate(shards, axis=0)  # [npad, f2]
    out = out_perm[prep["nid"][:cfg.n_real]]
    return out, res


def kernel(in_feat, edge_index, edge_weight, W1, b1, W2, b2):
    cfg = Cfg(n_real=100000, f_in=64, f_out=16, blocks_per_core=98)
    out, _ = run(cfg, in_feat, edge_index, edge_weight, W1, b1, W2, b2)
    return np.ascontiguousarray(out.astype(np.float32))


# revision 26
# speedup vs baseline: 1.2989x; 1.0133x over previous
"""Trainium2 Bass kernel for a 2-layer GCN (gnn_message_passing) — v3.

Strategy (8 NeuronCores, SPMD). Profile-driven rebuild of v2:
  - The v2 bottleneck was GPSIMD: dma_gather costs ~5-8ns/index + ~3us
    fixed of Pool-engine time (Q7 descriptor-gen), so v3 uses far fewer,
    far larger gather calls: K_CALLMAX=4096 (one call per run instead of
    ~8) -> 70 calls/layer instead of 266. Calls >1024 indices require
    single_packet=False: with single_packet=True the whole per-engine
    descriptor stream coalesces into ONE DMA packet, and 1024 idx x 256B
    / 16 engines = 16KB is the packet-size ceiling (bigger crashes).
  - PSUM: one bank per open dest block (HW allows only ONE live matmul
    accumulation group per bank — interleaving two groups in one bank
    drops accumulations; verified by micro-test), so SEPT=7 + head.
  - Tables stored bf16 padded to 256B rows ([N,128] bf16, features in
    cols 0..63). dma_gather descriptors then move 128B/row instead of
    256B (elem_size=64, elem_step=128 -> stride 256B satisfies the HW
    stride%256 constraint; the bass-level elem_size_bytes%256 assert is a
    transpose-mode restriction, bypassed by emitting InstDMAGatherAnt
    directly). Halves gather HBM traffic and kills the fp32->bf16 cast.
  - Scatter masks (weighted one-hot [slot x dest] bf16) are built on DVE
    (tensor_scalar is_eq*mult) and optionally a fraction on the idle
    Activation engine via two activations: y=Abs(iota-c) then
    mask=Relu(w - w*y)  (K_ACTMASK_PCT).
  - Dest nodes balance-permuted, sharded 8 x (98 blocks x 128); weights
    replicated; per-layer tables (dinv-prescaled transformed features)
    produced shard-wise and AllGathered; 'own' runs gather from the local
    shard table to hide AllGather latency.
"""

import sys
import types

if "/opt/trn_rl_repo" not in sys.path:
    sys.path.insert(0, "/opt/trn_rl_repo")

import numpy as np


def _install_ntff_shim():
    """antenv.axon_hooks is missing in this container; provide it so
    trace=True (NTFF profiling) works under axon."""
    if "antenv.axon_hooks" in sys.modules:
        return
    try:
        import antenv  # noqa: F401
    except ImportError:
        return
    shim = types.ModuleType("antenv.axon_hooks")
    shim._hook = None

    def set_axon_ntff_profile_hook(h):
        shim._hook = h

    def get_axon_ntff_profile_hook():
        return shim._hook

    shim.set_axon_ntff_profile_hook = set_axon_ntff_profile_hook
    shim.get_axon_ntff_profile_hook = get_axon_ntff_profile_hook
    sys.modules["antenv.axon_hooks"] = shim
    import antenv

    antenv.axon_hooks = shim
    try:
        from trn_agent_boot.trn_boot import _ntff_profile_via_ctypes

        shim._hook = _ntff_profile_via_ctypes("/opt/axon/libaxon_pjrt.so")
    except Exception:
        pass


import os

N_CORES = 8
P = 128
NB = 98                     # dest blocks per core
SHARD = NB * P              # 12544
NPAD = N_CORES * SHARD      # 100352
CROWS = NPAD // 4           # 25088 rows per global chunk (int16-safe)
QROWS = SHARD // 4          # 3136: shard quarter; chunk j is the
                            # rank-concat of all cores' quarter j, so one
                            # quarter-slice AllGather produces chunk j and
                            # global runs only wait for their own chunk
SEPT = int(os.environ.get("K_SEPT", "7"))    # blocks per dest group
# HW: only ONE live matmul accumulation group per PSUM bank (verified by
# micro-test: interleaved start/accum in one bank drops accumulations), so
# every open dest block needs its own bank: SEPT+head <= 8.
assert SEPT <= 7
NSEPT = NB // SEPT
assert NSEPT * SEPT == NB
NSRC = 5                    # source tables: 0=own shard, 1..4=global chunks
CALLMAX = int(os.environ.get("K_CALLMAX", "4096"))   # gather idxs per call
SCRATCH = int(os.environ.get("K_SCRATCH", "16384"))  # SWDGE ring bytes
NQUEUES = int(os.environ.get("K_NQUEUES", "1"))      # SWDGE queues
BF16TAB = int(os.environ.get("K_BF16TAB", "1"))      # bf16 padded tables
ACTPCT = int(os.environ.get("K_ACTMASK_PCT", "60"))  # % masks on Act engine
TPAD = 128                  # padded table row (bf16 elems) = 256B stride
OWN_FROM_FULL = False       # debug: route own edges via the global table


class Cfg:
    def __init__(self, n_real, f_in, f_out, blocks_per_core, sb_blocks=7):
        assert blocks_per_core == NB
        self.n_real = n_real
        self.f = f_in
        self.f2 = f_out
        self.nb = blocks_per_core
        self.shard = SHARD
        self.npad = NPAD
        assert self.npad >= n_real


def _host_prep(cfg, in_feat, edge_index, edge_weight):
    """Index/layout preprocessing (numpy only; all float math on device
    except copying edge weights into mask-scalar layouts)."""
    n, f = in_feat.shape
    assert n == cfg.n_real and f == cfg.f

    src = np.asarray(edge_index[0], dtype=np.int64)
    dst = np.asarray(edge_index[1], dtype=np.int64)
    ew = np.asarray(edge_weight, dtype=np.float32)
    loop = np.arange(n, dtype=np.int64)      # self loops, weight 1
    # Self-loops are NOT gathered per-edge: the self contribution is an
    # identity-mask matmul on the dest block's own table rows (affine DMA)
    # issued in finish_block. They still count for deg.
    dst_deg = np.concatenate([dst, loop])
    ew_deg = np.concatenate([ew, np.ones(n, np.float32)])

    # ---- balance-permute dest nodes into blocks of 128 slots -----------
    cnt = np.bincount(dst_deg, minlength=NPAD)
    order = np.argsort(-cnt, kind="stable")
    pos = np.arange(NPAD, dtype=np.int64)
    nblocks = N_CORES * NB
    pas = pos // nblocks
    bin_idx = pos % nblocks
    odd = (pas % 2) == 1
    bin_idx[odd] = nblocks - 1 - bin_idx[odd]
    nid = np.empty(NPAD, dtype=np.int64)
    nid[order] = bin_idx * P + pas

    dst_n = nid[dst]
    src_n = nid[src]
    dst_ndeg = nid[dst_deg]
    L = int(cnt.max())

    core_e = dst_n // SHARD
    b_e = (dst_n % SHARD) // P
    q_e = b_e // SEPT                        # dest group
    src_core = src_n // SHARD
    own = src_core == core_e
    if OWN_FROM_FULL:
        own[:] = False
    src_r = src_n % SHARD
    s_e = np.where(own, 0, src_r // QROWS + 1).astype(np.int64)
    tidx_e = np.where(own, src_r,
                      (src_n // SHARD) * QROWS + src_r % QROWS)

    # ---- max-over-cores segment sizes ----------------------------------
    cnt_bs = np.zeros((N_CORES, NB, NSRC), np.int64)
    np.add.at(cnt_bs, (core_e, b_e, s_e), 1)
    R = cnt_bs.max(axis=0)                   # [NB, NSRC]

    # ---- run / segment layout ------------------------------------------
    seg_off = np.zeros((NB, NSRC), np.int64)   # slot offset within run
    run_base_qs = np.zeros((NSEPT, NSRC), np.int64)
    run_meta = []                              # per ri: (q, s, base, nslots, calls)
    slot_base = 0
    for q in range(NSEPT):
        for s in range(NSRC):
            off = 0
            for b in range(q * SEPT, (q + 1) * SEPT):
                seg_off[b, s] = off
                off += R[b, s]
            nsl = -(-off // P) * P
            calls = []
            cb = 0
            while cb < nsl:
                nn = min(CALLMAX, nsl - cb)
                calls.append((slot_base + cb, nn))
                cb += nn
            run_base_qs[q, s] = slot_base
            run_meta.append((q, s, slot_base, nsl, calls))
            slot_base += nsl
    TOT = slot_base
    assert TOT % 16 == 0

    # ---- events: one mask+matmul per (run, 128-slot group, block) ------
    # Enumerated segment-major so per-edge event ids are arithmetic.
    first_eid = np.full((NB, NSRC), -1, np.int64)
    seg_g0 = np.zeros((NB, NSRC), np.int64)
    ev_ri = []
    ev_g = []
    ev_b = []
    for q in range(NSEPT):
        for s in range(NSRC):
            ri = q * NSRC + s
            for b in range(q * SEPT, (q + 1) * SEPT):
                if R[b, s] == 0:
                    continue
                g0 = seg_off[b, s] // P
                g1 = (seg_off[b, s] + R[b, s] - 1) // P
                first_eid[b, s] = len(ev_ri)
                seg_g0[b, s] = g0
                for g in range(g0, g1 + 1):
                    ev_ri.append(ri)
                    ev_g.append(g)
                    ev_b.append(b)
    NEV = len(ev_ri)
    ev_ri = np.array(ev_ri)
    ev_g = np.array(ev_g)
    ev_b = np.array(ev_b)

    ev_start = np.zeros(NEV, bool)
    ev_stop = np.zeros(NEV, bool)
    for b in range(NB):
        ss = [s for s in range(NSRC) if R[b, s] > 0]
        assert ss, f"block {b} has no edges?"
        ev_start[first_eid[b, ss[0]]] = True
        ls = ss[-1]
        g0 = seg_off[b, ls] // P
        g1 = (seg_off[b, ls] + R[b, ls] - 1) // P
        ev_stop[first_eid[b, ls] + (g1 - g0)] = True

    # program iteration order: by (run, group, block)
    evprog = np.lexsort((ev_b, ev_g, ev_ri))

    # ---- per-edge slot / event / partition -----------------------------
    segkey = (core_e * NB + b_e) * NSRC + s_e
    eorder = np.argsort(segkey, kind="stable")
    key_s = segkey[eorder]
    seg_start = np.searchsorted(key_s, np.arange(N_CORES * NB * NSRC))
    rank = np.arange(len(key_s)) - seg_start[key_s]
    core_s = core_e[eorder]
    b_s = b_e[eorder]
    s_s = s_e[eorder]
    q_s = q_e[eorder]
    dst_s = dst_n[eorder]
    ew_s = ew[eorder]
    tidx_s = tidx_e[eorder]

    slotrel = seg_off[b_s, s_s] + rank          # slot within run
    slot = run_base_qs[q_s, s_s] + slotrel      # global slot
    gi_run = slotrel // P
    p_slot = slotrel % P
    eid = first_eid[b_s, s_s] + (gi_run - seg_g0[b_s, s_s])

    colrel_t = np.zeros((N_CORES, P, NEV), np.float32)
    wsel_t = np.zeros((N_CORES, P, NEV), np.float32)
    colrel_t[core_s, p_slot, eid] = (dst_s % P).astype(np.float32)
    wsel_t[core_s, p_slot, eid] = ew_s

    # ---- int16 gather indices (16-partition wrap, replicated to 128) ---
    idx_cols = TOT // 16
    idx16 = np.zeros((N_CORES, 16, idx_cols), np.int16)
    idx16[core_s, slot % 16, slot // 16] = tidx_s.astype(np.int16)
    idx16 = np.broadcast_to(
        idx16[:, None, :, :], (N_CORES, 8, 16, idx_cols)
    ).reshape(N_CORES, P, idx_cols)

    # ---- deg layout (device computes deg = row-sum, dinv = 1/sqrt) -----
    dorder = np.argsort(dst_ndeg, kind="stable")
    dst_d = dst_ndeg[dorder]
    ew_d = ew_deg[dorder]
    dfirst = np.searchsorted(dst_d, np.arange(NPAD))
    rankd = np.arange(len(dst_d)) - dfirst[dst_d]
    assert rankd.max() < L
    ewdeg = np.zeros((N_CORES, P, NB * L), np.float32)
    ewdeg[dst_d // SHARD, dst_d % P, ((dst_d % SHARD) // P) * L + rankd] = ew_d
    zdeg = np.where(cnt == 0)[0]
    if len(zdeg) > 0:
        zn = nid[zdeg]
        ewdeg[zn // SHARD, zn % P, ((zn % SHARD) // P) * L] = 1.0

    # ---- permuted, padded, transposed features -------------------------
    xperm = np.zeros((NPAD, f), np.float32)
    xperm[nid[:n]] = np.asarray(in_feat, np.float32)
    xt_shards = [
        np.ascontiguousarray(xperm[c * SHARD:(c + 1) * SHARD].T)
        for c in range(N_CORES)
    ]

    return dict(
        L=L, nid=nid, xt_shards=xt_shards, NEV=NEV, TOT=TOT,
        colrel_t=colrel_t, wsel_t=wsel_t, ewdeg=ewdeg, idx16=idx16,
        idx_cols=idx_cols, run_meta=run_meta, evprog=evprog,
        ev_ri=ev_ri, ev_g=ev_g, ev_b=ev_b, ev_start=ev_start,
        ev_stop=ev_stop,
    )


def _emit_gather(nc, out_ap, in_ap, idxs_ap, num_idxs, elem_size,
                 elem_step, queue_num=0):
    """Emit InstDMAGatherAnt with elem_size_bytes that need not be a
    multiple of 256 (that bass-level assert is a transpose-mode
    restriction; the HW constraint is stride_bytes%256==0, which holds
    via elem_step). Mirrors BassGpSimd.dma_gather's lowering."""
    from concourse import mybir
    from concourse.ap_utils import ap_is_contiguous

    eng = nc.gpsimd
    assert idxs_ap.dtype == mybir.dt.int16
    assert in_ap.dtype == out_ap.dtype
    assert ap_is_contiguous(in_ap.ap[1:])
    assert ap_is_contiguous(out_ap.ap[1:])
    assert ap_is_contiguous(idxs_ap.ap[1:])
    assert in_ap.ap[-1][1] == out_ap.ap[-1][1] == elem_size
    assert out_ap.ap[0][1] * out_ap.ap[1][1] == -(-num_idxs // 128) * 128
    assert in_ap.ap[0][0] == elem_step
    stride_bytes = elem_step * mybir.dt.size(in_ap.dtype)
    assert stride_bytes % 256 == 0 and stride_bytes // 256 < 256
    _in_ap = eng.lower_ap_dma(in_ap, for_custom_bir_dma=True)
    _idxs_ap = eng.lower_ap(idxs_ap)
    _out_ap = eng.lower_ap(out_ap)
    return eng.add_instruction(
        mybir.InstDMAGatherAnt(
            name=nc.get_next_instruction_name(),
            ins=[
                *_in_ap,
                _idxs_ap,
                eng.lower_val_access(eng.to_reg(num_idxs)),
            ],
            outs=[_out_ap],
            transpose=False,
            num_idxs=num_idxs,
            elem_size=elem_size,
            stride_bytes_256=stride_bytes // 256,
            gen_mode=0,
            single_packet=False,
            queue_num=queue_num,
            sbuf_tokens_per_rank=0,
            sbuf_free_dim_per_rank=0,
            sbuf_free_dim_pad_per_rank=0,
            sbuf_byte_offset=0,
        )
    )


def _build_program(cfg, prep):
    from concourse import bacc, mybir, tile

    f, f2 = cfg.f, cfg.f2
    L, NEV, idx_cols = prep["L"], prep["NEV"], prep["idx_cols"]
    run_meta = prep["run_meta"]
    evprog = prep["evprog"]
    ev_ri, ev_g, ev_b = prep["ev_ri"], prep["ev_g"], prep["ev_b"]
    ev_start, ev_stop = prep["ev_start"], prep["ev_stop"]
    fp32 = mybir.dt.float32
    bf16 = mybir.dt.bfloat16
    Alu = mybir.AluOpType
    Act = mybir.ActivationFunctionType
    tdt = bf16 if BF16TAB else fp32
    trow = TPAD if BF16TAB else f

    # events grouped per (run, call)
    ev_by_call = {}
    for e in evprog:
        ri = int(ev_ri[e])
        g = int(ev_g[e])
        k = (g * P) // CALLMAX
        gc = g - k * (CALLMAX // P)
        ev_by_call.setdefault((ri, k), []).append(
            (gc, int(ev_b[e]), int(e), bool(ev_start[e]), bool(ev_stop[e]))
        )

    nc = bacc.Bacc("TRN2", target_bir_lowering=False, debug=False,
                   num_devices=N_CORES, dynamic_dma_scratch_size=SCRATCH,
                   num_swdge_queues=NQUEUES)

    xt_in = nc.dram_tensor("xt", [f, SHARD], fp32, kind="ExternalInput")
    w1_in = nc.dram_tensor("w1", [f, f], fp32, kind="ExternalInput")
    w2_in = nc.dram_tensor("w2", [f, f2], fp32, kind="ExternalInput")
    b1_in = nc.dram_tensor("b1r", [P, f], fp32, kind="ExternalInput")
    b2_in = nc.dram_tensor("b2r", [P, f2], fp32, kind="ExternalInput")
    idx_in = nc.dram_tensor("idx", [P, idx_cols], mybir.dt.int16,
                            kind="ExternalInput")
    colrel_in = nc.dram_tensor("colrel", [P, NEV], fp32, kind="ExternalInput")
    wsel_in = nc.dram_tensor("wsel", [P, NEV], fp32, kind="ExternalInput")
    ewdeg_in = nc.dram_tensor("ewdeg", [P, NB * L], fp32,
                              kind="ExternalInput")
    out_t = nc.dram_tensor("out", [SHARD, f2], fp32, kind="ExternalOutput")

    xw1_shard = nc.dram_tensor("xw1_shard", [SHARD, trow], tdt,
                               kind="Internal")
    xw1_full = nc.dram_tensor("xw1_full", [NPAD, trow], tdt, kind="Internal",
                              addr_space="Shared")
    h2_shard = nc.dram_tensor("h2_shard", [SHARD, trow], tdt, kind="Internal")
    h2_full = nc.dram_tensor("h2_full", [NPAD, trow], tdt, kind="Internal",
                             addr_space="Shared")

    rg = [list(range(N_CORES))]
    GW = CALLMAX // P          # max groups per call

    with tile.TileContext(nc) as tc:
        with tc.tile_pool(name="const", bufs=1) as cpool:
            # ---- constants ---------------------------------------------
            iota_i = cpool.tile([P, P], mybir.dt.int32, name="iota_i")
            nc.gpsimd.iota(iota_i[:], pattern=[[1, P]], base=0,
                           channel_multiplier=0)
            iota_b = cpool.tile([P, P], bf16, name="iota_b")
            nc.vector.tensor_copy(out=iota_b[:], in_=iota_i[:])
            pidx_i = cpool.tile([P, 1], mybir.dt.int32, name="pidx_i")
            nc.gpsimd.iota(pidx_i[:], pattern=[[0, 1]], base=0,
                           channel_multiplier=1)
            pidx_f = cpool.tile([P, 1], fp32, name="pidx_f")
            nc.vector.tensor_copy(out=pidx_f[:], in_=pidx_i[:])
            ident_b = cpool.tile([P, P], bf16, name="ident_b")
            nc.vector.tensor_scalar(out=ident_b[:], in0=iota_b[:],
                                    scalar1=pidx_f[:, 0:1], scalar2=None,
                                    op0=Alu.is_equal)
            w1_sb = cpool.tile([f, f], fp32, name="w1_sb")
            nc.sync.dma_start(out=w1_sb[:], in_=w1_in[:])
            w1_bf = cpool.tile([f, f], bf16, name="w1_bf")
            nc.vector.tensor_copy(out=w1_bf[:], in_=w1_sb[:])
            w2_sb = cpool.tile([f, f2], fp32, name="w2_sb")
            nc.sync.dma_start(out=w2_sb[:], in_=w2_in[:])
            w2_bf = cpool.tile([f, f2], bf16, name="w2_bf")
            nc.vector.tensor_copy(out=w2_bf[:], in_=w2_sb[:])
            b1_sb = cpool.tile([P, f], fp32, name="b1_sb")
            nc.sync.dma_start(out=b1_sb[:], in_=b1_in[:])
            b2_sb = cpool.tile([P, f2], fp32, name="b2_sb")
            nc.sync.dma_start(out=b2_sb[:], in_=b2_in[:])
            idx_sb = cpool.tile([P, idx_cols], mybir.dt.int16, name="idx_sb")
            nc.sync.dma_start(out=idx_sb[:], in_=idx_in[:])
            colrel_sb = cpool.tile([P, NEV], fp32, name="colrel_sb")
            nc.sync.dma_start(out=colrel_sb[:], in_=colrel_in[:])
            wsel_sb = cpool.tile([P, NEV], fp32, name="wsel_sb")
            nc.sync.dma_start(out=wsel_sb[:], in_=wsel_in[:])
            # negated copies for Act-engine masks: y=Abs(iota-c);
            # mask=Relu(negw*y + w)
            ncol_sb = cpool.tile([P, NEV], fp32, name="ncol_sb")
            nc.vector.tensor_scalar(out=ncol_sb[:], in0=colrel_sb[:],
                                    scalar1=-1.0, scalar2=None, op0=Alu.mult)
            nwsel_sb = cpool.tile([P, NEV], fp32, name="nwsel_sb")
            nc.vector.tensor_scalar(out=nwsel_sb[:], in0=wsel_sb[:],
                                    scalar1=-1.0, scalar2=None, op0=Alu.mult)
            dinv_sb = cpool.tile([P, NB], fp32, name="dinv_sb")
            out_stage = cpool.tile([P, NB * f2], fp32, name="out_stage")

            # ---- deg -> dinv (core-local) ------------------------------
            with tc.tile_pool(name="deg", bufs=1) as degp:
                ewdeg_sb = degp.tile([P, NB * L], fp32)
                nc.sync.dma_start(out=ewdeg_sb[:], in_=ewdeg_in[:])
                deg_sb = degp.tile([P, NB], fp32)
                for b in range(NB):
                    nc.vector.reduce_sum(
                        out=deg_sb[:, b:b + 1],
                        in_=ewdeg_sb[:, b * L:(b + 1) * L],
                        axis=mybir.AxisListType.X)
                sq_sb = degp.tile([P, NB], fp32)
                nc.scalar.activation(out=sq_sb[:], in_=deg_sb[:],
                                     func=Act.Sqrt)
                nc.vector.reciprocal(out=dinv_sb[:], in_=sq_sb[:])

            # ---- dense: table1 = dinv * (X @ W1) for my shard ----------
            SCH = 14
            with tc.tile_pool(name="xt", bufs=1) as xtp, \
                 tc.tile_pool(name="dps", bufs=4, space="PSUM") as dpp, \
                 tc.tile_pool(name="dst", bufs=2) as dstp:
                xtf = xtp.tile([f, SHARD], fp32, name="xtf")
                nc.sync.dma_start(out=xtf[:], in_=xt_in[:])
                xtb = xtp.tile([f, SHARD], bf16, name="xtb")
                nc.scalar.activation(out=xtb[:], in_=xtf[:], func=Act.Copy)
                for t0 in range(0, NB, SCH):
                    stg = dstp.tile([P, SCH * f], tdt, tag="dstg",
                                    name="dstg")
                    for j in range(SCH):
                        b = t0 + j
                        ps = dpp.tile([P, f], fp32, tag="dps", name="dps")
                        nc.tensor.matmul(out=ps[:],
                                         lhsT=xtb[:, b * P:(b + 1) * P],
                                         rhs=w1_bf[:], start=True, stop=True)
                        nc.vector.tensor_scalar(
                            out=stg[:, j * f:(j + 1) * f], in0=ps[:],
                            scalar1=dinv_sb[:, b:b + 1], scalar2=None,
                            op0=Alu.mult)
                    dst_ap = xw1_shard[t0 * P:(t0 + SCH) * P, :f].rearrange(
                        "(i p) f -> p i f", p=P)
                    nc.sync.dma_start(out=dst_ap, in_=stg[:])

            for j in range(4):
                nc.gpsimd.collective_compute(
                    "AllGather", Alu.bypass, replica_groups=rg,
                    ins=[xw1_shard[j * QROWS:(j + 1) * QROWS, :]],
                    outs=[xw1_full[j * CROWS:(j + 1) * CROWS, :]])

            # ---- aggregation layers ------------------------------------
            with tc.tile_pool(name="gst", bufs=6) as gpool, \
                 tc.tile_pool(name="gbf", bufs=3) as bpool, \
                 tc.tile_pool(name="mask", bufs=16) as mpool, \
                 tc.tile_pool(name="work", bufs=4) as wpool:

                def agg_layer(layer, shard_t, full_t):
                    # One PSUM bank per open dest block (HW allows only one
                    # live accumulation group per bank), <=7 open + head.
                    pss = {}                  # b -> psum tile
                    callno = [0]
                    evno = [0]

                    def finish_block(b):
                        ps = pss.pop(b)
                        # self-loop: identity-mask matmul on this block's
                        # own table rows (affine DMA, no gather slot)
                        srow = wpool.tile([P, f], tdt, tag="srow",
                                          name="srow")
                        nc.sync.dma_start(
                            out=srow[:],
                            in_=shard_t[b * P:(b + 1) * P, :f])
                        if layer == 1:
                            nc.tensor.matmul(out=ps[:], lhsT=ident_b[:],
                                             rhs=srow[:], start=False,
                                             stop=True)
                        else:
                            nc.tensor.matmul(out=ps[:], lhsT=srow[:],
                                             rhs=ident_b[:], start=False,
                                             stop=True)
                        if layer == 1:
                            pblk = ps[:]
                            t1 = wpool.tile([P, f], fp32, tag="t1",
                                            name="t1")
                            nc.vector.tensor_scalar(
                                out=t1[:], in0=pblk,
                                scalar1=dinv_sb[:, b:b + 1], scalar2=None,
                                op0=Alu.mult)
                            t2 = wpool.tile([P, f], fp32, tag="t2",
                                            name="t2")
                            nc.vector.tensor_tensor(
                                out=t2[:], in0=t1[:], in1=b1_sb[:],
                                op=Alu.add)
                            h2w = wpool.tile([P, f], tdt, tag="h2w",
                                             name="h2w")
                            # dinv>0 so relu(dinv*x) == dinv*relu(x)
                            nc.scalar.activation(
                                out=h2w[:], in_=t2[:], func=Act.Relu,
                                scale=dinv_sb[:, b:b + 1])
                            nc.sync.dma_start(
                                out=h2_shard[b * P:(b + 1) * P, :f],
                                in_=h2w[:])
                        else:
                            pblk = ps[:]
                            lh = wpool.tile([f, P], bf16, tag="lh",
                                            name="lh")
                            nc.vector.tensor_copy(out=lh[:], in_=pblk)
                            ps2 = pph.tile([P, f2], fp32, tag="head",
                                           name="ps2")
                            nc.tensor.matmul(out=ps2[:], lhsT=lh[:],
                                             rhs=w2_bf[:], start=True,
                                             stop=True)
                            t3 = wpool.tile([P, f2], fp32, tag="t3",
                                            name="t3")
                            nc.vector.tensor_scalar(
                                out=t3[:], in0=ps2[:],
                                scalar1=dinv_sb[:, b:b + 1], scalar2=None,
                                op0=Alu.mult)
                            nc.vector.tensor_tensor(
                                out=out_stage[:, b * f2:(b + 1) * f2],
                                in0=t3[:], in1=b2_sb[:], op=Alu.add)

                    def build_mask(e):
                        mask = mpool.tile([P, P], bf16, tag="mask",
                                          name="mask")
                        evno[0] += 1
                        if (evno[0] * ACTPCT) // 100 != \
                           ((evno[0] - 1) * ACTPCT) // 100:
                            # Activation-engine build (2 ops)
                            ytmp = mpool.tile([P, P], bf16, tag="ytmp",
                                              name="ytmp")
                            nc.scalar.activation(
                                out=ytmp[:], in_=iota_b[:], func=Act.Abs,
                                bias=ncol_sb[:, e:e + 1])
                            nc.scalar.activation(
                                out=mask[:], in_=ytmp[:], func=Act.Relu,
                                scale=nwsel_sb[:, e:e + 1],
                                bias=wsel_sb[:, e:e + 1])
                        else:
                            nc.vector.tensor_scalar(
                                out=mask[:], in0=iota_b[:],
                                scalar1=colrel_sb[:, e:e + 1],
                                scalar2=wsel_sb[:, e:e + 1],
                                op0=Alu.is_equal, op1=Alu.mult)
                        return mask

                    for ri, (q, s, base, nsl, calls) in enumerate(run_meta):
                        if s == 0:
                            table = shard_t[:, :f]
                        else:
                            table = full_t[(s - 1) * CROWS:s * CROWS, :f]
                        for k, (cbase, nn) in enumerate(calls):
                            gt = gpool.tile([P, GW * f], tdt, tag="gst",
                                            name=f"gt{layer}_{ri}_{k}")
                            if BF16TAB:
                                _emit_gather(
                                    nc,
                                    gt[:, :nn // P * f].rearrange(
                                        "p (a q) -> p a q", q=f),
                                    table,
                                    idx_sb[:, cbase // 16:(cbase + nn) // 16],
                                    nn, f, TPAD,
                                    queue_num=callno[0] % NQUEUES)
                            else:
                                nc.gpsimd.dma_gather(
                                    gt[:, :nn // P * f].rearrange(
                                        "p (a q) -> p a q", q=f),
                                    table,
                                    idx_sb[:, cbase // 16:(cbase + nn) // 16],
                                    nn, nn, f,
                                    single_packet=False,
                                    queue_num=callno[0] % NQUEUES)
                            callno[0] += 1
                            if BF16TAB:
                                gb_t = gt
                            else:
                                gb_t = bpool.tile([P, GW * f], bf16,
                                                  tag="gbf",
                                                  name=f"gb{layer}_{ri}_{k}")
                                nc.scalar.activation(
                                    out=gb_t[:, :nn // P * f],
                                    in_=gt[:, :nn // P * f], func=Act.Copy)
                            for gc, b, e, st, sp in ev_by_call.get(
                                    (ri, k), []):
                                mask = build_mask(e)
                                msg = gb_t[:, gc * f:(gc + 1) * f]
                                if b not in pss:
                                    if layer == 1:
                                        pss[b] = pp.tile([P, f], fp32,
                                                         tag="agg",
                                                         name="aps")
                                    else:
                                        pss[b] = pp.tile([f, P], fp32,
                                                         tag="agg",
                                                         name="apsT")
                                    st = True
                                if layer == 1:
                                    nc.tensor.matmul(out=pss[b][:],
                                                     lhsT=mask[:], rhs=msg,
                                                     start=st, stop=False)
                                else:
                                    nc.tensor.matmul(out=pss[b][:],
                                                     lhsT=msg, rhs=mask[:],
                                                     start=st, stop=False)
                                if sp:
                                    finish_block(b)
                    assert not pss, list(pss)

                with tc.tile_pool(name="agg1", bufs=SEPT,
                                  space="PSUM") as pp:
                    agg_layer(1, xw1_shard, xw1_full)

                for j in range(4):
                    nc.gpsimd.collective_compute(
                        "AllGather", Alu.bypass, replica_groups=rg,
                        ins=[h2_shard[j * QROWS:(j + 1) * QROWS, :]],
                        outs=[h2_full[j * CROWS:(j + 1) * CROWS, :]])

                with tc.tile_pool(name="agg2", bufs=SEPT,
                                  space="PSUM") as pp, \
                     tc.tile_pool(name="head", bufs=1, space="PSUM") as pph:
                    agg_layer(2, h2_shard, h2_full)

            out_ap = out_t[:].rearrange("(b p) f -> p b f", p=P)
            nc.sync.dma_start(out=out_ap, in_=out_stage[:])

    nc.compile()
    return nc


def _make_in_maps(cfg, prep, W1, b1, W2, b2):
    b1r = np.broadcast_to(np.asarray(b1, np.float32), (P, cfg.f)).copy()
    b2r = np.broadcast_to(np.asarray(b2, np.float32), (P, cfg.f2)).copy()
    w1 = np.asarray(W1, np.float32)
    w2 = np.asarray(W2, np.float32)
    in_maps = []
    for c in range(N_CORES):
        in_maps.append({
            "xt": prep["xt_shards"][c],
            "w1": w1, "w2": w2, "b1r": b1r, "b2r": b2r,
            "idx": np.ascontiguousarray(prep["idx16"][c]),
            "colrel": np.ascontiguousarray(prep["colrel_t"][c]),
            "wsel": np.ascontiguousarray(prep["wsel_t"][c]),
            "ewdeg": np.ascontiguousarray(prep["ewdeg"][c]),
        })
    return in_maps


def run(cfg, in_feat, edge_index, edge_weight, W1, b1, W2, b2,
        trace=False, use_sim=False):
    """Returns (output [n_real, f2], BassKernelResults|None)."""
    _install_ntff_shim()
    from concourse import bass_utils

    prep = _host_prep(cfg, in_feat, edge_index, edge_weight)
    nc = _build_program(cfg, prep)
    in_maps = _make_in_maps(cfg, prep, W1, b1, W2, b2)

    if use_sim:
        from concourse.bass_interp import MultiCoreSim
        sim = MultiCoreSim(nc, num_cores=N_CORES)
        for c, (cid, core) in enumerate(sim.cores.items()):
            for k, v in in_maps[c].items():
                core.tensor(k)[:] = v
        sim.simulate()
        shards = [sim.cores[c].tensor("out").copy() for c in sim.cores]
        res = None
    else:
        res = bass_utils.run_bass_kernel_spmd(
            nc, in_maps, core_ids=list(range(N_CORES)), trace=trace)
        shards = [res.results[c]["out"] for c in range(N_CORES)]

    out_perm = np.concatenate(shards, axis=0)  # [npad, f2]
    out = out_perm[prep["nid"][:cfg.n_real]]
    return out, res


def kernel(in_feat, edge_index, edge_weight, W1, b1, W2, b2):
    cfg = Cfg(n_real=100000, f_in=64, f_out=16, blocks_per_core=98)
    out, _ = run(cfg, in_feat, edge_index, edge_weight, W1, b1, W2, b2)
    return np.ascontiguousarray(out.astype(np.float32))
